# revision 34
# baseline (speedup 1.0000x reference)
"""Trainium2 Bass kernel for windowed multi-head attention (Swin-style).

Problem: B=4096 windows x N=64 tokens x C=128 channels, H=4 heads, hd=32.
  qkv = x @ w_qkv ; attn = softmax(q k^T / sqrt(hd) + rel_bias) ; out = (attn v) @ w_proj + b

Sharding: data-parallel over windows, 512 windows per core on 8 cores.

Dataflow (per superchunk of 4 windows = 256 tokens; matmul operands bf16):
  x downcast on GPSIMD; xT via PE bf16 transpose; qT/kT weight-stationary (512 cols)
  v[m, c-perm] head-parity packed (512 cols, no duplication)
  attnT[m, n] per (window, head) (1024 cols)
  P = exp(attnT) (ACT) * exp(bias) (DVE bf16)
  av[n, hd] with P^T stationary (512 cols) + softmax sums as 1-col matmuls (16)
  av_sb = av * recip(s) broadcast (DVE, fused evacuation)
  avT via 8 PE [64,64] transposes (512 cols); y = avT @ wp_perm (256 cols)
  b_proj added on HOST after gather.

The per-superchunk stages are emitted SOFTWARE-PIPELINED with a deep skew;
at iteration i the emitted stage instances are
  ATT(i-2) DC(i+3) T(i+2) XTE(i+1) QKV(i) QKE/VE(i-1) EXP(i-3) BIAS(i-4)
  AVS(i-6) NORM(i-7) AVTE(i-9) AVT(i-8) PROJ(i-10) YE(i-11) RECIP(i-6)
so every cross-engine dependency (except the deliberate tail RECIP) is at
least one iteration old and no engine head-of-line blocks. Engine split:
PE matmuls ~1.50us/iter, ACT {qk-evac, exp, y-evac} ~1.63us, DVE {xt/v/avT
evacs, recip, norm} ~1.43us, Pool (GPSIMD, no PSUM access!) {x downcast,
bias mul} ~1.56us, DMA ~0.75us. x is group-prefetched ~1.5 groups ahead.

PSUM-bank drain rule respected: no two in-flight matmuls with different
tile_position row-groups and the same column-group share a PSUM bank.
"""

import sys

sys.path.insert(0, "/opt/trn_rl_repo")

import numpy as np
import ml_dtypes

WS = 8
H = 4
DIM = 128
N = WS * WS  # 64 tokens per window
HD = DIM // H  # 32
B = 4096
NCORES = 8
BC = B // NCORES  # 512 windows per core
ROWS = BC * N  # 32768 rows per core

SC_W = 4  # windows per superchunk
SC_ROWS = SC_W * N  # 256
N_SC = BC // SC_W  # 128 superchunks
GROUP = 4  # superchunks per input DMA group
N_G = N_SC // GROUP  # 32 groups

bf16 = ml_dtypes.bfloat16

# channel permutation used for wv cols / wp rows / avT partitions:
# p = 64*hp + 32*hh + d  <->  c = 32*(2*hh + hp) + d   (head h = 2*hh + hp)
_PERM = np.array(
    [32 * (2 * ((p % 64) // 32) + p // 64) + p % 32 for p in range(DIM)], dtype=np.int64
)


def _rel_pos_index(ws: int) -> np.ndarray:
    coords = np.stack(np.meshgrid(np.arange(ws), np.arange(ws), indexing="ij"))
    flat = coords.reshape(2, -1)
    rel = flat[:, :, None] - flat[:, None, :]
    rel = rel.transpose(1, 2, 0).astype(np.int64)
    rel[..., 0] += ws - 1
    rel[..., 1] += ws - 1
    rel[..., 0] *= 2 * ws - 1
    return rel.sum(-1)


_BUILT = {}


def _build_program(n_groups=N_G, compile=True, stage=9, bufs=None):
    """Build + compile the single-core Bass program (same program runs SPMD
    on all cores). stage < 9 truncates the pipeline for bisection: that
    stage's intermediate is written to y instead.

    See module docstring for the stage skew; measured 266545 ns/core in
    TimelineSim (baseline kernel: 485740 ns)."""
    bufs = dict(
        dict(sb1=7, sb2=5, qk=2, xp=3, yp=4, att_first=1),
        **(bufs or {}),
    )
    key = (n_groups, compile, stage, tuple(sorted(bufs.items())))
    if key in _BUILT:
        return _BUILT[key]
    n_sc = n_groups * GROUP

    from contextlib import ExitStack

    import concourse.tile as tile
    from concourse import bacc, mybir
    from concourse.masks import make_identity

    f32 = mybir.dt.float32
    bf = mybir.dt.bfloat16
    EXP = mybir.ActivationFunctionType.Exp

    nc = bacc.Bacc("TRN2", target_bir_lowering=False, debug=False, enable_asserts=False)

    x_d = nc.dram_tensor("x", [n_sc * SC_ROWS, DIM], f32, kind="ExternalInput").ap()
    wq_d = nc.dram_tensor("wq_bf", [DIM, DIM], bf, kind="ExternalInput").ap()
    wk_d = nc.dram_tensor("wk_bf", [DIM, DIM], bf, kind="ExternalInput").ap()
    wv_d = nc.dram_tensor("wvp_bf", [DIM, DIM], bf, kind="ExternalInput").ap()
    wp_d = nc.dram_tensor("wpp_bf", [DIM, DIM], bf, kind="ExternalInput").ap()
    # eb[64*hp + m, 256*hb + 64*w + n] = exp(rel_bias[2*hb+hp][n, m]), window-tiled
    eb_d = nc.dram_tensor("expbias_bf", [DIM, 2 * H * N], bf, kind="ExternalInput").ap()
    y_d = nc.dram_tensor("y", [n_sc * SC_ROWS, DIM], f32, kind="ExternalOutput").ap()

    with tile.TileContext(nc) as tc, ExitStack() as ctx:
        consts = ctx.enter_context(tc.tile_pool(name="consts", bufs=1))
        xp = ctx.enter_context(tc.tile_pool(name="xp", bufs=bufs["xp"]))
        sb1 = ctx.enter_context(tc.tile_pool(name="sb1", bufs=bufs["sb1"]))
        sb2 = ctx.enter_context(tc.tile_pool(name="sb2", bufs=bufs["sb2"]))
        yp = ctx.enter_context(tc.tile_pool(name="yp", bufs=bufs["yp"]))

        # PSUM (8 banks of 2KB/partition):
        #   at  [128,1024] f32, manual parity halves        2 banks
        #   qk  [128,512]  f32 x2 bufs                      2 banks
        #   v   [128,256]  f32 x2                           1 bank
        #   av  [128,256]  f32 x2                           1 bank
        #   y   [128,256]  f32 x2                           1 bank
        #   sm: xt_ps bf16 x2 + avT_ps bf16 x1 + s f32 x2   1 bank
        ps_at = ctx.enter_context(tc.tile_pool(name="ps_at", bufs=1, space="PSUM"))
        ps_qk = ctx.enter_context(tc.tile_pool(name="ps_qk", bufs=bufs["qk"], space="PSUM"))
        ps_v = ctx.enter_context(tc.tile_pool(name="ps_v", bufs=1, space="PSUM"))
        ps_av = ctx.enter_context(tc.tile_pool(name="ps_av", bufs=1, space="PSUM"))
        ps_y = ctx.enter_context(tc.tile_pool(name="ps_y", bufs=1, space="PSUM"))
        ps_sm = ctx.enter_context(tc.tile_pool(name="ps_sm", bufs=1, space="PSUM"))

        # constants
        wq = consts.tile([DIM, DIM], bf)
        wk = consts.tile([DIM, DIM], bf)
        wvp = consts.tile([DIM, DIM], bf)
        wpp = consts.tile([DIM, DIM], bf)
        eb = consts.tile([DIM, 2 * H * N], bf)
        nc.sync.dma_start(wq[:], wq_d)
        nc.sync.dma_start(wk[:], wk_d)
        nc.sync.dma_start(wvp[:], wv_d)
        nc.sync.dma_start(wpp[:], wp_d)
        nc.sync.dma_start(eb[:], eb_d)

        identf = consts.tile([DIM, DIM], f32)
        make_identity(nc, identf[:])
        ident = consts.tile([DIM, DIM], bf)
        nc.gpsimd.tensor_copy(ident[:], identf[:])
        ones1 = consts.tile([DIM, 1], bf)
        nc.vector.memset(ones1[:], 1.0)

        live = {}
        cur_ps = {}
        cur_ysb = [None]

        def x_load(g):
            x_g = xp.tile([128, 2 * GROUP, DIM], f32, tag="x_g")
            r0 = g * GROUP * SC_ROWS
            nc.sync.dma_start(
                out=x_g[:],
                in_=x_d[r0 : r0 + GROUP * SC_ROWS, :].rearrange(
                    "(t p) c -> p t c", p=128
                ),
            )
            return x_g

        def st_DC(i):
            """Pool: downcast x -> bf16."""
            sc = live[i]
            so = i % GROUP
            x_bf = sb1.tile([128, 2, DIM], bf, tag="x_bf")
            nc.gpsimd.tensor_copy(x_bf[:], sc["x_g"][:, 2 * so : 2 * so + 2, :])
            sc["x_bf"] = x_bf

        def st_T(i):
            """PE: bf16 transpose -> xt_ps."""
            sc = live[i]
            xt_ps = cur_ps["sm"][:, 128 * (i % 2) : 128 * (i % 2) + 128].bitcast(bf)
            for t in range(2):
                nc.tensor.matmul(
                    xt_ps[:, t * 128 : (t + 1) * 128],
                    sc["x_bf"][:, t, :],
                    ident[:],
                    is_transpose=True,
                    start=True,
                    stop=True,
                )
            sc["xt_ps"] = xt_ps

        def st_XTE(i):
            """DVE: xt evac (bf16 2x)."""
            sc = live[i]
            xt = sb1.tile([128, SC_ROWS], bf, tag="xt")
            nc.vector.tensor_copy(xt[:], sc["xt_ps"][:])
            sc["xt"] = xt

        def st_QKV(i):
            sc = live[i]
            xt = sc["xt"]
            qk_ps = ps_qk.tile([128, 2 * SC_ROWS], f32, tag="qk_ps")
            nc.tensor.matmul(qk_ps[:, 0:SC_ROWS], wq[:], xt[:], start=True, stop=True)
            nc.tensor.matmul(
                qk_ps[:, SC_ROWS : 2 * SC_ROWS], wk[:], xt[:], start=True, stop=True
            )
            v_ps = ps_v.tile([128, 2 * SC_ROWS], f32, tag="v_ps", name="v_ps")[
                :, 256 * (i % 2) : 256 * (i % 2) + SC_ROWS
            ]
            for w in range(SC_W):
                for hp in range(2):
                    nc.tensor.matmul(
                        v_ps[64 * hp : 64 * hp + 64, 64 * w : 64 * (w + 1)],
                        xt[:, 64 * w : 64 * (w + 1)],
                        wvp[:, 64 * hp : 64 * hp + 64],
                        tile_position=(0, 64 * hp),
                        start=True,
                        stop=True,
                    )
            sc["qk_ps"] = qk_ps
            sc["v_ps"] = v_ps

        def st_QKE(i):
            sc = live[i]
            qk = sb1.tile([128, 2 * SC_ROWS], bf, tag="qk")
            nc.scalar.copy(qk[:], sc["qk_ps"][:])
            sc["qk"] = qk

        def st_VE(i):
            sc = live[i]
            vd = sb1.tile([128, SC_ROWS], bf, tag="vd")
            nc.vector.tensor_copy(vd[:], sc["v_ps"][:])
            sc["vd"] = vd

        def st_ATT(i):
            """PE attnT (at_ps halves alternate by sc parity); ACT exp; Pool bias."""
            sc = live[i]
            qk = sc["qk"]
            at_ps = ps_at.tile([128, 1024], f32, tag="at_ps")
            off = 256 * (i % 2)
            for w in range(SC_W):
                for h in range(H):
                    hp, hb = h % 2, h // 2
                    nc.tensor.matmul(
                        at_ps[
                            64 * hp : 64 * hp + 64,
                            512 * hb + off + 64 * w : 512 * hb + off + 64 * (w + 1),
                        ],
                        qk[32 * h : 32 * h + 32, SC_ROWS + 64 * w : SC_ROWS + 64 * (w + 1)],
                        qk[32 * h : 32 * h + 32, 64 * w : 64 * (w + 1)],
                        tile_position=(32 * h, 64 * hp),
                        start=True,
                        stop=True,
                    )
            sc["at_ps"] = at_ps
            sc["at_off"] = off

        def st_EXP(i):
            sc = live[i]
            at_ps, off = sc["at_ps"], sc["at_off"]
            pt = sb2.tile([128, 2 * SC_ROWS], bf, tag="pt")
            nc.scalar.activation(
                pt[:].rearrange("p (b c) -> p b c", b=2),
                at_ps[:].rearrange("p (b c) -> p b c", b=2)[:, :, off : off + SC_ROWS],
                EXP,
            )
            sc["pt"] = pt

        def st_BIAS(i):
            sc = live[i]
            pb = sb2.tile([128, 2 * SC_ROWS], bf, tag="pb")
            if bufs.get("bias_split"):
                nc.vector.tensor_mul(
                    pb[:, 0:SC_ROWS], sc["pt"][:, 0:SC_ROWS], eb[:, 0:SC_ROWS]
                )
                nc.gpsimd.tensor_mul(
                    pb[:, SC_ROWS : 2 * SC_ROWS],
                    sc["pt"][:, SC_ROWS : 2 * SC_ROWS],
                    eb[:, SC_ROWS : 2 * SC_ROWS],
                )
            else:
                eng = nc.vector if bufs.get("bias_dve") else nc.gpsimd
                eng.tensor_mul(pb[:], sc["pt"][:], eb[:, 0 : 2 * SC_ROWS])
            sc["pb"] = pb

        def st_AVS(i):
            """PE: av[n, hd] + sums (P^T stationary)."""
            sc = live[i]
            pb, vd = sc["pb"], sc["vd"]
            av_ps = ps_av.tile([128, 2 * SC_ROWS], f32, tag="av_ps", name="av_ps")[
                :, 256 * (i % 2) : 256 * (i % 2) + SC_ROWS
            ]
            s_ps = cur_ps["sm"][:, 384 + 8 * (i % 2) : 384 + 8 * (i % 2) + 8]
            for w in range(SC_W):
                for h in range(H):
                    hp, hh = h % 2, h // 2
                    pbl = pb[
                        64 * hp : 64 * hp + 64,
                        256 * hh + 64 * w : 256 * hh + 64 * (w + 1),
                    ]
                    nc.tensor.matmul(
                        av_ps[
                            64 * hp : 64 * hp + 64,
                            64 * w + 32 * hh : 64 * w + 32 * hh + 32,
                        ],
                        pbl,
                        vd[
                            64 * hp : 64 * hp + 64,
                            64 * w + 32 * hh : 64 * w + 32 * hh + 32,
                        ],
                        tile_position=(64 * hp, 64 * hp),
                        start=True,
                        stop=True,
                    )
                    nc.tensor.matmul(
                        s_ps[64 * hp : 64 * hp + 64, 2 * w + hh : 2 * w + hh + 1],
                        pbl,
                        ones1[64 * hp : 64 * hp + 64, :],
                        tile_position=(64 * hp, 64 * hp),
                        start=True,
                        stop=True,
                    )
            sc["av_ps"] = av_ps
            sc["s_ps"] = s_ps

        def st_RECIP(i):
            sc = live[i]
            rf = sb2.tile([128, 8], f32, tag="rf")
            nc.vector.reciprocal_approx_fast(out=rf[:], in_=sc["s_ps"][:])
            sc["rf"] = rf

        def st_NORM(i):
            sc = live[i]
            av_sb = sb2.tile([128, SC_ROWS], bf, tag="av_sb")
            rf_b = sc["rf"][:].unsqueeze(-1).broadcast_to((128, 8, 32))
            nc.vector.tensor_mul(
                av_sb[:].rearrange("p (k d) -> p k d", k=8),
                sc["av_ps"][:].rearrange("p (k d) -> p k d", k=8),
                rf_b,
            )
            sc["av_sb"] = av_sb

        def st_AVT(i):
            sc = live[i]
            av_sb = sc["av_sb"]
            avT_ps = cur_ps["sm"][:, 256:384].bitcast(bf)
            for w in range(SC_W):
                for hp in range(2):
                    nc.tensor.matmul(
                        avT_ps[64 * hp : 64 * hp + 64, 64 * w : 64 * (w + 1)],
                        av_sb[64 * hp : 64 * hp + 64, 64 * w : 64 * (w + 1)],
                        ident[64 * hp : 64 * hp + 64, 64 * hp : 64 * hp + 64],
                        is_transpose=True,
                        tile_position=(64 * hp, 64 * hp),
                        start=True,
                        stop=True,
                    )
            sc["avT_ps"] = avT_ps

        def st_AVTE(i):
            sc = live[i]
            avt = sb1.tile([128, SC_ROWS], bf, tag="avt")
            nc.vector.tensor_copy(avt[:], sc["avT_ps"][:])
            sc["avt"] = avt

        def st_PROJ(i):
            sc = live[i]
            avt = sc["avt"]
            y_tile = ps_y.tile([128, 4 * DIM], f32, tag="y_ps", name="y_ps")
            y_ps = y_tile[:, 256 * (i % 2) : 256 * (i % 2) + 2 * DIM]
            sc["y_tile"] = y_tile
            for j in range(2):
                nc.tensor.matmul(
                    y_ps[:, 128 * j : 128 * (j + 1)],
                    avt[:, 128 * j : 128 * (j + 1)],
                    wpp[:],
                    start=True,
                    stop=True,
                )
            sc["y_ps"] = y_ps

        def st_YE(i):
            sc = live[i]
            half = i % 2
            if half == 0:
                cur_ysb[0] = yp.tile([128, 4, DIM], f32, tag="y_sb", name="y_sb")
            y_sb = cur_ysb[0]
            nc.scalar.copy(
                y_sb[:, 2 * half : 2 * half + 2, :].rearrange("p a b -> p (a b)"),
                sc["y_ps"][:],
            )
            if half == 1:
                r0 = (i - 1) * SC_ROWS
                nc.sync.dma_start(
                    out=y_d[r0 : r0 + 2 * SC_ROWS, :].rearrange(
                        "(t p) c -> p t c", p=128
                    ),
                    in_=y_sb[:],
                )

        def dump(i, src_ap):
            """Bisect helper (SBUF src only): route [128, 256] to y rows of sc i."""
            half = i % 2
            if half == 0:
                cur_ysb[0] = yp.tile([128, 4, DIM], f32, tag="y_sb", name="y_sb")
            y_sb = cur_ysb[0]
            nc.gpsimd.tensor_copy(
                y_sb[:, 2 * half : 2 * half + 2, :].rearrange("p a b -> p (a b)"),
                src_ap,
            )
            if half == 1:
                r0 = (i - 1) * SC_ROWS
                nc.sync.dma_start(
                    out=y_d[r0 : r0 + 2 * SC_ROWS, :].rearrange(
                        "(t p) c -> p t c", p=128
                    ),
                    in_=y_sb[:],
                )

        def run_iter(it, last):
            cur_ps["sm"] = ps_sm.tile([128, 512], f32, tag="sm", name="sm")
            if bufs.get("att_first") and 0 <= it - 2 <= last and stage >= 3:
                st_ATT(it - 2)
            if it + 7 <= last + 4 and (it + 7) % GROUP == 0:
                g = (it + 7) // GROUP
                if 0 < g < n_groups:
                    live_g = x_load(g)
                    for k in range(GROUP):
                        live.setdefault(g * GROUP + k, {})["x_g"] = live_g
            if 0 <= it + 3 <= last:
                st_DC(it + 3)
            if 0 <= it + 2 <= last:
                st_T(it + 2)
            if 0 <= it + 1 <= last:
                st_XTE(it + 1)
                if stage < 2:
                    dump(it + 1, live[it + 1]["xt"][:])
            if 0 <= it <= last and stage >= 2:
                st_QKV(it)
            if bufs.get("exp_first") and 0 <= it - 3 <= last and stage >= 3:
                st_EXP(it - 3)
            if 0 <= it - 1 <= last and stage >= 2:
                st_QKE(it - 1)
                st_VE(it - 1)
                if stage < 3:
                    dump(it - 1, live[it - 1]["qk"][:, 0:SC_ROWS])
            if not bufs.get("att_first") and 0 <= it - 2 <= last and stage >= 3:
                st_ATT(it - 2)
            if not bufs.get("exp_first") and 0 <= it - 3 <= last and stage >= 3:
                st_EXP(it - 3)
            bo = 5 if bufs.get("bias_deep") else 4
            if 0 <= it - bo <= last and stage >= 3:
                st_BIAS(it - bo)
                if stage < 4:
                    dump(it - bo, live[it - bo]["pb"][:, 0:SC_ROWS])
            if 0 <= it - 6 <= last and stage >= 4:
                st_AVS(it - 6)
            if bufs.get("recip7") and 0 <= it - 7 <= last and stage >= 4:
                st_RECIP(it - 7)
            if 0 <= it - 7 <= last and stage >= 4:
                st_NORM(it - 7)
                if stage < 5:
                    dump(it - 7, live[it - 7]["av_sb"][:])
            td = 1 if bufs.get("tail_deep") else 0
            if 0 <= it - 9 - td <= last and stage >= 5:
                st_AVTE(it - 9 - td)
            if 0 <= it - 8 - td <= last and stage >= 5:
                st_AVT(it - 8 - td)
            if 0 <= it - 10 - td <= last and stage >= 5:
                st_PROJ(it - 10 - td)
            if 0 <= it - 11 - td <= last and stage >= 5:
                st_YE(it - 11 - td)
                live.pop(it - 11 - td)
            if not bufs.get("recip7") and 0 <= it - 6 <= last and stage >= 4:
                st_RECIP(it - 6)

        g0 = x_load(0)
        for k in range(GROUP):
            live.setdefault(k, {})["x_g"] = g0
        for it in range(-3, n_sc + 12 + (1 if bufs.get("tail_deep") else 0)):
            run_iter(it, n_sc - 1)

    if compile:
        nc.compile()
    _BUILT[key] = nc
    return nc


def _host_prep(w_qkv, w_proj, bias_table):
    """Precompute replicated small tensors (channel-permuted for the kernel)."""
    scale = HD**-0.5
    wq = (w_qkv[:, :DIM] * scale).astype(bf16)
    wk = w_qkv[:, DIM : 2 * DIM].astype(bf16)
    wv = w_qkv[:, 2 * DIM :]
    wvp = np.ascontiguousarray(wv[:, _PERM]).astype(bf16)
    wpp = np.ascontiguousarray(w_proj[_PERM, :]).astype(bf16)

    rel = _rel_pos_index(WS)  # [N, N]
    rel_bias = bias_table[rel.reshape(-1)].reshape(N, N, H).transpose(2, 0, 1)  # [h,n,m]
    ebv = np.exp(rel_bias).astype(np.float32)  # [h, n, m]
    # eb[64*hp + m, 256*hb + 64*w + n] = ebv[2*hb + hp][n, m]
    eb = np.zeros((DIM, 512), np.float32)
    for hb in range(2):
        for hp in range(2):
            h = 2 * hb + hp
            blk = ebv[h].T  # [m, n]
            for w in range(SC_W):
                eb[
                    64 * hp : 64 * hp + 64, 256 * hb + 64 * w : 256 * hb + 64 * (w + 1)
                ] = blk
    eb = eb.astype(bf16)
    return wq, wk, wvp, wpp, eb


def run(x, w_qkv, w_proj, b_proj, bias_table, trace=False, **trace_kwargs):
    """Run on 8 NeuronCores. Returns (y, BassKernelResults)."""
    from concourse import bass_utils

    x = np.asarray(x, dtype=np.float32)
    w_qkv = np.asarray(w_qkv, dtype=np.float32)
    w_proj = np.asarray(w_proj, dtype=np.float32)
    b_proj = np.asarray(b_proj, dtype=np.float32)
    bias_table = np.asarray(bias_table, dtype=np.float32)

    wq, wk, wvp, wpp, eb = _host_prep(w_qkv, w_proj, bias_table)
    nc = _build_program()

    xs = x.reshape(B * N, DIM)
    in_maps = []
    for c in range(NCORES):
        in_maps.append(
            {
                "x": np.ascontiguousarray(xs[c * ROWS : (c + 1) * ROWS]),
                "wq_bf": wq,
                "wk_bf": wk,
                "wvp_bf": wvp,
                "wpp_bf": wpp,
                "expbias_bf": eb,
            }
        )

    res = bass_utils.run_bass_kernel_spmd(
        nc, in_maps, core_ids=list(range(NCORES)), trace=trace, **trace_kwargs
    )
    y = np.concatenate([res.results[c]["y"] for c in range(NCORES)], axis=0)
    y = y + b_proj[None, :]
    return y.reshape(B, N, DIM), res


def kernel(x, w_qkv, w_proj, b_proj, bias_table):
    y, _ = run(x, w_qkv, w_proj, b_proj, bias_table)
    return y


if __name__ == "__main__":
    sys.path.insert(0, "/root/problem")
    import reference

    inputs = {k: np.asarray(v) for k, v in reference.setup_inputs().items()}
    out = kernel(**inputs)
    exp = np.asarray(reference.reference(**inputs))
    err = np.abs(out - exp)
    print("abs max err:", err.max(), "scale-rel:", err.max() / np.abs(exp).max())


# revision 37
# speedup vs baseline: 1.0094x; 1.0094x over previous
"""Trainium2 Bass kernel for windowed multi-head attention (Swin-style).

Problem: B=4096 windows x N=64 tokens x C=128 channels, H=4 heads, hd=32.
  qkv = x @ w_qkv ; attn = softmax(q k^T / sqrt(hd) + rel_bias) ; out = (attn v) @ w_proj + b

Sharding: data-parallel over windows, 512 windows per core on 8 cores.

Dataflow (per superchunk of 4 windows = 256 tokens; matmul operands bf16):
  x downcast on GPSIMD; xT via PE bf16 transpose; qT/kT weight-stationary (512 cols)
  v[m, c-perm] head-parity packed (512 cols, no duplication)
  attnT[m, n] per (window, head) (1024 cols)
  P = exp(attnT) (ACT) * exp(bias) (DVE bf16)
  av[n, hd] with P^T stationary (512 cols) + softmax sums as 1-col matmuls (16)
  av_sb = av * recip(s) broadcast (DVE, fused evacuation)
  avT via 8 PE [64,64] transposes (512 cols); y = avT @ wp_perm (256 cols)
  b_proj added on HOST after gather.

The per-superchunk stages are emitted SOFTWARE-PIPELINED with a deep skew;
at iteration i the emitted stage instances are
  ATT(i-2) DC(i+3) T(i+2) XTE(i+1) QKV(i) QKE/VE(i-1) EXP(i-3) BIAS(i-4)
  AVS(i-6) NORM(i-7) AVTE(i-9) AVT(i-8) PROJ(i-10) YE(i-11) RECIP(i-6)
so every cross-engine dependency (except the deliberate tail RECIP) is at
least one iteration old and no engine head-of-line blocks. Engine split:
PE matmuls ~1.50us/iter, ACT {qk-evac, exp, y-evac} ~1.63us, DVE {xt/v/avT
evacs, recip, norm} ~1.43us, Pool (GPSIMD, no PSUM access!) {x downcast,
bias mul} ~1.56us, DMA ~0.75us. x is group-prefetched ~1.5 groups ahead.

PSUM-bank drain rule respected: no two in-flight matmuls with different
tile_position row-groups and the same column-group share a PSUM bank.
"""

import sys

sys.path.insert(0, "/opt/trn_rl_repo")

import numpy as np
import ml_dtypes

WS = 8
H = 4
DIM = 128
N = WS * WS  # 64 tokens per window
HD = DIM // H  # 32
B = 4096
NCORES = 8
BC = B // NCORES  # 512 windows per core
ROWS = BC * N  # 32768 rows per core

SC_W = 4  # windows per superchunk
SC_ROWS = SC_W * N  # 256
N_SC = BC // SC_W  # 128 superchunks
GROUP = 4  # superchunks per input DMA group
N_G = N_SC // GROUP  # 32 groups

bf16 = ml_dtypes.bfloat16

# channel permutation used for wv cols / wp rows / avT partitions:
# p = 64*hp + 32*hh + d  <->  c = 32*(2*hh + hp) + d   (head h = 2*hh + hp)
_PERM = np.array(
    [32 * (2 * ((p % 64) // 32) + p // 64) + p % 32 for p in range(DIM)], dtype=np.int64
)


def _rel_pos_index(ws: int) -> np.ndarray:
    coords = np.stack(np.meshgrid(np.arange(ws), np.arange(ws), indexing="ij"))
    flat = coords.reshape(2, -1)
    rel = flat[:, :, None] - flat[:, None, :]
    rel = rel.transpose(1, 2, 0).astype(np.int64)
    rel[..., 0] += ws - 1
    rel[..., 1] += ws - 1
    rel[..., 0] *= 2 * ws - 1
    return rel.sum(-1)


_BUILT = {}


def _build_program(n_groups=N_G, compile=True, stage=9, bufs=None):
    """Build + compile the single-core Bass program (same program runs SPMD
    on all cores). stage < 9 truncates the pipeline for bisection: that
    stage's intermediate is written to y instead.

    See module docstring for the stage skew; measured 264067 ns/core in
    TimelineSim (baseline kernel: 485740 ns)."""
    bufs = dict(
        dict(sb1=7, sb2=5, qk=2, xp=3, yp=4, att_first=1, hoist=1),
        **(bufs or {}),
    )
    key = (n_groups, compile, stage, tuple(sorted(bufs.items())))
    if key in _BUILT:
        return _BUILT[key]
    n_sc = n_groups * GROUP

    from contextlib import ExitStack

    import concourse.tile as tile
    from concourse import bacc, mybir
    from concourse.masks import make_identity

    f32 = mybir.dt.float32
    bf = mybir.dt.bfloat16
    EXP = mybir.ActivationFunctionType.Exp

    nc = bacc.Bacc("TRN2", target_bir_lowering=False, debug=False, enable_asserts=False)

    x_d = nc.dram_tensor("x", [n_sc * SC_ROWS, DIM], f32, kind="ExternalInput").ap()
    wq_d = nc.dram_tensor("wq_bf", [DIM, DIM], bf, kind="ExternalInput").ap()
    wk_d = nc.dram_tensor("wk_bf", [DIM, DIM], bf, kind="ExternalInput").ap()
    wv_d = nc.dram_tensor("wvp_bf", [DIM, DIM], bf, kind="ExternalInput").ap()
    wp_d = nc.dram_tensor("wpp_bf", [DIM, DIM], bf, kind="ExternalInput").ap()
    # eb[64*hp + m, 256*hb + 64*w + n] = exp(rel_bias[2*hb+hp][n, m]), window-tiled
    eb_d = nc.dram_tensor("expbias_bf", [DIM, 2 * H * N], bf, kind="ExternalInput").ap()
    y_d = nc.dram_tensor("y", [n_sc * SC_ROWS, DIM], f32, kind="ExternalOutput").ap()

    with tile.TileContext(nc) as tc, ExitStack() as ctx:
        consts = ctx.enter_context(tc.tile_pool(name="consts", bufs=1))
        xp = ctx.enter_context(tc.tile_pool(name="xp", bufs=bufs["xp"]))
        sb1 = ctx.enter_context(tc.tile_pool(name="sb1", bufs=bufs["sb1"]))
        sb2 = ctx.enter_context(tc.tile_pool(name="sb2", bufs=bufs["sb2"]))
        yp = ctx.enter_context(tc.tile_pool(name="yp", bufs=bufs["yp"]))

        # PSUM (8 banks of 2KB/partition):
        #   at  [128,1024] f32, manual parity halves        2 banks
        #   qk  [128,512]  f32 x2 bufs                      2 banks
        #   v   [128,256]  f32 x2                           1 bank
        #   av  [128,256]  f32 x2                           1 bank
        #   y   [128,256]  f32 x2                           1 bank
        #   sm: xt_ps bf16 x2 + avT_ps bf16 x1 + s f32 x2   1 bank
        ps_at = ctx.enter_context(tc.tile_pool(name="ps_at", bufs=1, space="PSUM"))
        ps_qk = ctx.enter_context(tc.tile_pool(name="ps_qk", bufs=bufs["qk"], space="PSUM"))
        ps_v = ctx.enter_context(tc.tile_pool(name="ps_v", bufs=1, space="PSUM"))
        ps_av = ctx.enter_context(tc.tile_pool(name="ps_av", bufs=1, space="PSUM"))
        ps_y = ctx.enter_context(tc.tile_pool(name="ps_y", bufs=1, space="PSUM"))
        ps_sm = ctx.enter_context(tc.tile_pool(name="ps_sm", bufs=1, space="PSUM"))

        # constants
        wq = consts.tile([DIM, DIM], bf)
        wk = consts.tile([DIM, DIM], bf)
        wvp = consts.tile([DIM, DIM], bf)
        wpp = consts.tile([DIM, DIM], bf)
        eb = consts.tile([DIM, 2 * H * N], bf)
        nc.sync.dma_start(wq[:], wq_d)
        nc.sync.dma_start(wk[:], wk_d)
        nc.sync.dma_start(wvp[:], wv_d)
        nc.sync.dma_start(wpp[:], wp_d)
        nc.sync.dma_start(eb[:], eb_d)

        identf = consts.tile([DIM, DIM], f32)
        make_identity(nc, identf[:])
        ident = consts.tile([DIM, DIM], bf)
        nc.gpsimd.tensor_copy(ident[:], identf[:])
        ones1 = consts.tile([DIM, 1], bf)
        nc.vector.memset(ones1[:], 1.0)

        live = {}
        cur_ps = {}
        cur_ysb = [None]

        def x_load(g):
            x_g = xp.tile([128, 2 * GROUP, DIM], f32, tag="x_g")
            r0 = g * GROUP * SC_ROWS
            nc.sync.dma_start(
                out=x_g[:],
                in_=x_d[r0 : r0 + GROUP * SC_ROWS, :].rearrange(
                    "(t p) c -> p t c", p=128
                ),
            )
            return x_g

        def st_DC(i):
            """Pool: downcast x -> bf16."""
            sc = live[i]
            so = i % GROUP
            x_bf = sb1.tile([128, 2, DIM], bf, tag="x_bf")
            nc.gpsimd.tensor_copy(x_bf[:], sc["x_g"][:, 2 * so : 2 * so + 2, :])
            sc["x_bf"] = x_bf

        def st_T(i):
            """PE: bf16 transpose -> xt_ps."""
            sc = live[i]
            xt_ps = cur_ps["sm"][:, 128 * (i % 2) : 128 * (i % 2) + 128].bitcast(bf)
            for t in range(2):
                nc.tensor.matmul(
                    xt_ps[:, t * 128 : (t + 1) * 128],
                    sc["x_bf"][:, t, :],
                    ident[:],
                    is_transpose=True,
                    start=True,
                    stop=True,
                )
            sc["xt_ps"] = xt_ps

        def st_XTE(i):
            """DVE: xt evac (bf16 2x)."""
            sc = live[i]
            xt = sb1.tile([128, SC_ROWS], bf, tag="xt")
            nc.vector.tensor_copy(xt[:], sc["xt_ps"][:])
            sc["xt"] = xt

        def st_QKV(i):
            sc = live[i]
            xt = sc["xt"]
            qk_ps = ps_qk.tile([128, 2 * SC_ROWS], f32, tag="qk_ps")
            nc.tensor.matmul(qk_ps[:, 0:SC_ROWS], wq[:], xt[:], start=True, stop=True)
            nc.tensor.matmul(
                qk_ps[:, SC_ROWS : 2 * SC_ROWS], wk[:], xt[:], start=True, stop=True
            )
            v_ps = ps_v.tile([128, 2 * SC_ROWS], f32, tag="v_ps", name="v_ps")[
                :, 256 * (i % 2) : 256 * (i % 2) + SC_ROWS
            ]
            for w in range(SC_W):
                for hp in range(2):
                    nc.tensor.matmul(
                        v_ps[64 * hp : 64 * hp + 64, 64 * w : 64 * (w + 1)],
                        xt[:, 64 * w : 64 * (w + 1)],
                        wvp[:, 64 * hp : 64 * hp + 64],
                        tile_position=(0, 64 * hp),
                        start=True,
                        stop=True,
                    )
            sc["qk_ps"] = qk_ps
            sc["v_ps"] = v_ps

        def st_QKE(i):
            sc = live[i]
            qk = sb1.tile([128, 2 * SC_ROWS], bf, tag="qk")
            nc.scalar.copy(qk[:], sc["qk_ps"][:])
            sc["qk"] = qk

        def st_VE(i):
            sc = live[i]
            vd = sb1.tile([128, SC_ROWS], bf, tag="vd")
            nc.vector.tensor_copy(vd[:], sc["v_ps"][:])
            sc["vd"] = vd

        def st_ATT(i):
            """PE attnT (at_ps halves alternate by sc parity); ACT exp; Pool bias."""
            sc = live[i]
            qk = sc["qk"]
            at_ps = ps_at.tile([128, 1024], f32, tag="at_ps")
            off = 256 * (i % 2)
            for w in range(SC_W):
                for h in range(H):
                    hp, hb = h % 2, h // 2
                    nc.tensor.matmul(
                        at_ps[
                            64 * hp : 64 * hp + 64,
                            512 * hb + off + 64 * w : 512 * hb + off + 64 * (w + 1),
                        ],
                        qk[32 * h : 32 * h + 32, SC_ROWS + 64 * w : SC_ROWS + 64 * (w + 1)],
                        qk[32 * h : 32 * h + 32, 64 * w : 64 * (w + 1)],
                        tile_position=(32 * h, 64 * hp),
                        start=True,
                        stop=True,
                    )
            sc["at_ps"] = at_ps
            sc["at_off"] = off

        def st_EXP(i):
            sc = live[i]
            at_ps, off = sc["at_ps"], sc["at_off"]
            pt = sb2.tile([128, 2 * SC_ROWS], bf, tag="pt")
            nc.scalar.activation(
                pt[:].rearrange("p (b c) -> p b c", b=2),
                at_ps[:].rearrange("p (b c) -> p b c", b=2)[:, :, off : off + SC_ROWS],
                EXP,
            )
            sc["pt"] = pt

        def st_BIAS(i):
            sc = live[i]
            pb = sb2.tile([128, 2 * SC_ROWS], bf, tag="pb")
            if bufs.get("bias_split"):
                nc.vector.tensor_mul(
                    pb[:, 0:SC_ROWS], sc["pt"][:, 0:SC_ROWS], eb[:, 0:SC_ROWS]
                )
                nc.gpsimd.tensor_mul(
                    pb[:, SC_ROWS : 2 * SC_ROWS],
                    sc["pt"][:, SC_ROWS : 2 * SC_ROWS],
                    eb[:, SC_ROWS : 2 * SC_ROWS],
                )
            else:
                eng = nc.vector if bufs.get("bias_dve") else nc.gpsimd
                eng.tensor_mul(pb[:], sc["pt"][:], eb[:, 0 : 2 * SC_ROWS])
            sc["pb"] = pb

        def st_AVS(i):
            """PE: av[n, hd] + sums (P^T stationary)."""
            sc = live[i]
            pb, vd = sc["pb"], sc["vd"]
            av_ps = ps_av.tile([128, 2 * SC_ROWS], f32, tag="av_ps", name="av_ps")[
                :, 256 * (i % 2) : 256 * (i % 2) + SC_ROWS
            ]
            s_ps = cur_ps["sm"][:, 384 + 8 * (i % 2) : 384 + 8 * (i % 2) + 8]
            for w in range(SC_W):
                for h in range(H):
                    hp, hh = h % 2, h // 2
                    pbl = pb[
                        64 * hp : 64 * hp + 64,
                        256 * hh + 64 * w : 256 * hh + 64 * (w + 1),
                    ]
                    nc.tensor.matmul(
                        av_ps[
                            64 * hp : 64 * hp + 64,
                            64 * w + 32 * hh : 64 * w + 32 * hh + 32,
                        ],
                        pbl,
                        vd[
                            64 * hp : 64 * hp + 64,
                            64 * w + 32 * hh : 64 * w + 32 * hh + 32,
                        ],
                        tile_position=(64 * hp, 64 * hp),
                        start=True,
                        stop=True,
                    )
                    nc.tensor.matmul(
                        s_ps[64 * hp : 64 * hp + 64, 2 * w + hh : 2 * w + hh + 1],
                        pbl,
                        ones1[64 * hp : 64 * hp + 64, :],
                        tile_position=(64 * hp, 64 * hp),
                        start=True,
                        stop=True,
                    )
            sc["av_ps"] = av_ps
            sc["s_ps"] = s_ps

        def st_RECIP(i):
            sc = live[i]
            rf = sb2.tile([128, 8], f32, tag="rf")
            nc.vector.reciprocal_approx_fast(out=rf[:], in_=sc["s_ps"][:])
            sc["rf"] = rf

        def st_NORM(i):
            sc = live[i]
            av_sb = sb2.tile([128, SC_ROWS], bf, tag="av_sb")
            rf_b = sc["rf"][:].unsqueeze(-1).broadcast_to((128, 8, 32))
            nc.vector.tensor_mul(
                av_sb[:].rearrange("p (k d) -> p k d", k=8),
                sc["av_ps"][:].rearrange("p (k d) -> p k d", k=8),
                rf_b,
            )
            sc["av_sb"] = av_sb

        def st_AVT(i):
            sc = live[i]
            av_sb = sc["av_sb"]
            avT_ps = cur_ps["sm"][:, 256:384].bitcast(bf)
            for w in range(SC_W):
                for hp in range(2):
                    nc.tensor.matmul(
                        avT_ps[64 * hp : 64 * hp + 64, 64 * w : 64 * (w + 1)],
                        av_sb[64 * hp : 64 * hp + 64, 64 * w : 64 * (w + 1)],
                        ident[64 * hp : 64 * hp + 64, 64 * hp : 64 * hp + 64],
                        is_transpose=True,
                        tile_position=(64 * hp, 64 * hp),
                        start=True,
                        stop=True,
                    )
            sc["avT_ps"] = avT_ps

        def st_AVTE(i):
            sc = live[i]
            avt = sb1.tile([128, SC_ROWS], bf, tag="avt")
            nc.vector.tensor_copy(avt[:], sc["avT_ps"][:])
            sc["avt"] = avt

        def st_PROJ(i):
            sc = live[i]
            avt = sc["avt"]
            y_tile = ps_y.tile([128, 4 * DIM], f32, tag="y_ps", name="y_ps")
            y_ps = y_tile[:, 256 * (i % 2) : 256 * (i % 2) + 2 * DIM]
            sc["y_tile"] = y_tile
            for j in range(2):
                nc.tensor.matmul(
                    y_ps[:, 128 * j : 128 * (j + 1)],
                    avt[:, 128 * j : 128 * (j + 1)],
                    wpp[:],
                    start=True,
                    stop=True,
                )
            sc["y_ps"] = y_ps

        def st_YE(i):
            sc = live[i]
            half = i % 2
            if half == 0:
                cur_ysb[0] = yp.tile([128, 4, DIM], f32, tag="y_sb", name="y_sb")
            y_sb = cur_ysb[0]
            nc.scalar.copy(
                y_sb[:, 2 * half : 2 * half + 2, :].rearrange("p a b -> p (a b)"),
                sc["y_ps"][:],
            )
            if half == 1:
                r0 = (i - 1) * SC_ROWS
                nc.sync.dma_start(
                    out=y_d[r0 : r0 + 2 * SC_ROWS, :].rearrange(
                        "(t p) c -> p t c", p=128
                    ),
                    in_=y_sb[:],
                )

        def dump(i, src_ap):
            """Bisect helper (SBUF src only): route [128, 256] to y rows of sc i."""
            half = i % 2
            if half == 0:
                cur_ysb[0] = yp.tile([128, 4, DIM], f32, tag="y_sb", name="y_sb")
            y_sb = cur_ysb[0]
            nc.gpsimd.tensor_copy(
                y_sb[:, 2 * half : 2 * half + 2, :].rearrange("p a b -> p (a b)"),
                src_ap,
            )
            if half == 1:
                r0 = (i - 1) * SC_ROWS
                nc.sync.dma_start(
                    out=y_d[r0 : r0 + 2 * SC_ROWS, :].rearrange(
                        "(t p) c -> p t c", p=128
                    ),
                    in_=y_sb[:],
                )

        def run_iter(it, last):
            cur_ps["sm"] = ps_sm.tile([128, 512], f32, tag="sm", name="sm")
            td = 1 if bufs.get("tail_deep") else 0
            bo = 5 if bufs.get("bias_deep") else 4

            def S(cond, fn, *a):
                if cond:
                    fn(*a)

            if bufs.get("hoist2"):
                # consumers-first: every op whose deps are >=1 iter old is
                # emitted before this iteration's producers.
                S(0 <= it - 9 - td <= last and stage >= 5, st_AVTE, it - 9 - td)
                S(0 <= it - 7 <= last and stage >= 4, st_NORM, it - 7)
                S(0 <= it + 1 <= last, st_XTE, it + 1)
                S(0 <= it - 1 <= last and stage >= 2, st_QKE, it - 1)
                S(0 <= it - 3 <= last and stage >= 3, st_EXP, it - 3)
                S(0 <= it - 11 - td <= last and stage >= 5, st_YE, it - 11 - td)
                S(0 <= it - bo <= last and stage >= 3, st_BIAS, it - bo)
                if it + 7 <= last + 4 and (it + 7) % GROUP == 0:
                    g = (it + 7) // GROUP
                    if 0 < g < n_groups:
                        live_g = x_load(g)
                        for k in range(GROUP):
                            live.setdefault(g * GROUP + k, {})["x_g"] = live_g
                S(0 <= it + 3 <= last, st_DC, it + 3)
                S(0 <= it - 2 <= last and stage >= 3, st_ATT, it - 2)
                S(0 <= it + 2 <= last, st_T, it + 2)
                S(0 <= it <= last and stage >= 2, st_QKV, it)
                S(0 <= it - 6 <= last and stage >= 4, st_AVS, it - 6)
                S(0 <= it - 8 - td <= last and stage >= 5, st_AVT, it - 8 - td)
                S(0 <= it - 10 - td <= last and stage >= 5, st_PROJ, it - 10 - td)
                S(0 <= it - 1 <= last and stage >= 2, st_VE, it - 1)
                S(0 <= it - 6 <= last and stage >= 4, st_RECIP, it - 6)
                if stage < 2 and 0 <= it + 1 <= last:
                    dump(it + 1, live[it + 1]["xt"][:])
                if stage == 2 and 0 <= it - 1 <= last:
                    dump(it - 1, live[it - 1]["qk"][:, 0:SC_ROWS])
                if stage == 3 and 0 <= it - bo <= last:
                    dump(it - bo, live[it - bo]["pb"][:, 0:SC_ROWS])
                if stage == 4 and 0 <= it - 7 <= last:
                    dump(it - 7, live[it - 7]["av_sb"][:])
                if 0 <= it - 12 - td <= last and stage >= 5:
                    live.pop(it - 12 - td)
                return

            if bufs.get("hoist"):
                td0 = 1 if bufs.get("tail_deep") else 0
                if 0 <= it - 9 - td0 <= last and stage >= 5:
                    st_AVTE(it - 9 - td0)
                if 0 <= it - 7 <= last and stage >= 4:
                    st_NORM(it - 7)
            if bufs.get("att_first") and 0 <= it - 2 <= last and stage >= 3:
                st_ATT(it - 2)
            if it + 7 <= last + 4 and (it + 7) % GROUP == 0:
                g = (it + 7) // GROUP
                if 0 < g < n_groups:
                    live_g = x_load(g)
                    for k in range(GROUP):
                        live.setdefault(g * GROUP + k, {})["x_g"] = live_g
            if 0 <= it + 3 <= last:
                st_DC(it + 3)
            if 0 <= it + 2 <= last:
                st_T(it + 2)
            if 0 <= it + 1 <= last:
                st_XTE(it + 1)
                if stage < 2:
                    dump(it + 1, live[it + 1]["xt"][:])
            if 0 <= it <= last and stage >= 2:
                st_QKV(it)
            if 0 <= it - 1 <= last and stage >= 2:
                st_QKE(it - 1)
                st_VE(it - 1)
                if stage < 3:
                    dump(it - 1, live[it - 1]["qk"][:, 0:SC_ROWS])
            if not bufs.get("att_first") and 0 <= it - 2 <= last and stage >= 3:
                st_ATT(it - 2)
            if 0 <= it - 3 <= last and stage >= 3:
                st_EXP(it - 3)
            if 0 <= it - bo <= last and stage >= 3:
                st_BIAS(it - bo)
                if stage < 4:
                    dump(it - bo, live[it - bo]["pb"][:, 0:SC_ROWS])
            if 0 <= it - 6 <= last and stage >= 4:
                st_AVS(it - 6)
            if 0 <= it - 7 <= last and stage >= 4:
                if not bufs.get("hoist"):
                    st_NORM(it - 7)
                if stage < 5:
                    dump(it - 7, live[it - 7]["av_sb"][:])
            if not bufs.get("hoist") and 0 <= it - 9 - td <= last and stage >= 5:
                st_AVTE(it - 9 - td)
            if 0 <= it - 8 - td <= last and stage >= 5:
                st_AVT(it - 8 - td)
            if 0 <= it - 10 - td <= last and stage >= 5:
                st_PROJ(it - 10 - td)
            if 0 <= it - 11 - td <= last and stage >= 5:
                st_YE(it - 11 - td)
                live.pop(it - 11 - td)
            if not bufs.get("recip7") and 0 <= it - 6 <= last and stage >= 4:
                st_RECIP(it - 6)

        g0 = x_load(0)
        for k in range(GROUP):
            live.setdefault(k, {})["x_g"] = g0
        for it in range(-3, n_sc + 12 + (1 if bufs.get("tail_deep") else 0)):
            run_iter(it, n_sc - 1)

    if compile:
        nc.compile()
    _BUILT[key] = nc
    return nc


def _host_prep(w_qkv, w_proj, bias_table):
    """Precompute replicated small tensors (channel-permuted for the kernel)."""
    scale = HD**-0.5
    wq = (w_qkv[:, :DIM] * scale).astype(bf16)
    wk = w_qkv[:, DIM : 2 * DIM].astype(bf16)
    wv = w_qkv[:, 2 * DIM :]
    wvp = np.ascontiguousarray(wv[:, _PERM]).astype(bf16)
    wpp = np.ascontiguousarray(w_proj[_PERM, :]).astype(bf16)

    rel = _rel_pos_index(WS)  # [N, N]
    rel_bias = bias_table[rel.reshape(-1)].reshape(N, N, H).transpose(2, 0, 1)  # [h,n,m]
    ebv = np.exp(rel_bias).astype(np.float32)  # [h, n, m]
    # eb[64*hp + m, 256*hb + 64*w + n] = ebv[2*hb + hp][n, m]
    eb = np.zeros((DIM, 512), np.float32)
    for hb in range(2):
        for hp in range(2):
            h = 2 * hb + hp
            blk = ebv[h].T  # [m, n]
            for w in range(SC_W):
                eb[
                    64 * hp : 64 * hp + 64, 256 * hb + 64 * w : 256 * hb + 64 * (w + 1)
                ] = blk
    eb = eb.astype(bf16)
    return wq, wk, wvp, wpp, eb


def run(x, w_qkv, w_proj, b_proj, bias_table, trace=False, **trace_kwargs):
    """Run on 8 NeuronCores. Returns (y, BassKernelResults)."""
    from concourse import bass_utils

    x = np.asarray(x, dtype=np.float32)
    w_qkv = np.asarray(w_qkv, dtype=np.float32)
    w_proj = np.asarray(w_proj, dtype=np.float32)
    b_proj = np.asarray(b_proj, dtype=np.float32)
    bias_table = np.asarray(bias_table, dtype=np.float32)

    wq, wk, wvp, wpp, eb = _host_prep(w_qkv, w_proj, bias_table)
    nc = _build_program()

    xs = x.reshape(B * N, DIM)
    in_maps = []
    for c in range(NCORES):
        in_maps.append(
            {
                "x": np.ascontiguousarray(xs[c * ROWS : (c + 1) * ROWS]),
                "wq_bf": wq,
                "wk_bf": wk,
                "wvp_bf": wvp,
                "wpp_bf": wpp,
                "expbias_bf": eb,
            }
        )

    res = bass_utils.run_bass_kernel_spmd(
        nc, in_maps, core_ids=list(range(NCORES)), trace=trace, **trace_kwargs
    )
    y = np.concatenate([res.results[c]["y"] for c in range(NCORES)], axis=0)
    y = y + b_proj[None, :]
    return y.reshape(B, N, DIM), res


def kernel(x, w_qkv, w_proj, b_proj, bias_table):
    y, _ = run(x, w_qkv, w_proj, b_proj, bias_table)
    return y


if __name__ == "__main__":
    sys.path.insert(0, "/root/problem")
    import reference

    inputs = {k: np.asarray(v) for k, v in reference.setup_inputs().items()}
    out = kernel(**inputs)
    exp = np.asarray(reference.reference(**inputs))
    err = np.abs(out - exp)
    print("abs max err:", err.max(), "scale-rel:", err.max() / np.abs(exp).max())


# revision 42
# speedup vs baseline: 1.0097x; 1.0003x over previous
"""Trainium2 Bass kernel for windowed multi-head attention (Swin-style).

Problem: B=4096 windows x N=64 tokens x C=128 channels, H=4 heads, hd=32.
  qkv = x @ w_qkv ; attn = softmax(q k^T / sqrt(hd) + rel_bias) ; out = (attn v) @ w_proj + b

Sharding: data-parallel over windows, 512 windows per core on 8 cores.

Dataflow (per superchunk of 4 windows = 256 tokens; matmul operands bf16):
  x downcast on GPSIMD; xT via PE bf16 transpose; qT/kT weight-stationary (512 cols)
  v[m, c-perm] head-parity packed (512 cols, no duplication)
  attnT[m, n] per (window, head) (1024 cols)
  P = exp(attnT) (ACT) * exp(bias) (DVE bf16)
  av[n, hd] with P^T stationary (512 cols) + softmax sums as 1-col matmuls (16)
  av_sb = av * recip(s) broadcast (DVE, fused evacuation)
  avT via 8 PE [64,64] transposes (512 cols); y = avT @ wp_perm (256 cols)
  b_proj added on HOST after gather.

The per-superchunk stages are emitted SOFTWARE-PIPELINED with a deep skew;
at iteration i the emitted stage instances are
  ATT(i-2) DC(i+3) T(i+2) XTE(i+1) QKV(i) QKE/VE(i-1) EXP(i-3) BIAS(i-4)
  AVS(i-6) NORM(i-7) AVTE(i-9) AVT(i-8) PROJ(i-10) YE(i-11) RECIP(i-6)
so every cross-engine dependency (except the deliberate tail RECIP) is at
least one iteration old and no engine head-of-line blocks. Engine split:
PE matmuls ~1.50us/iter, ACT {qk-evac, exp, y-evac} ~1.63us, DVE {xt/v/avT
evacs, recip, norm} ~1.43us, Pool (GPSIMD, no PSUM access!) {x downcast,
bias mul} ~1.56us, DMA ~0.75us. x is group-prefetched ~1.5 groups ahead.

PSUM-bank drain rule respected: no two in-flight matmuls with different
tile_position row-groups and the same column-group share a PSUM bank.
"""

import sys

sys.path.insert(0, "/opt/trn_rl_repo")

import numpy as np
import ml_dtypes

WS = 8
H = 4
DIM = 128
N = WS * WS  # 64 tokens per window
HD = DIM // H  # 32
B = 4096
NCORES = 8
BC = B // NCORES  # 512 windows per core
ROWS = BC * N  # 32768 rows per core

SC_W = 4  # windows per superchunk
SC_ROWS = SC_W * N  # 256
N_SC = BC // SC_W  # 128 superchunks
GROUP = 4  # superchunks per input DMA group
N_G = N_SC // GROUP  # 32 groups

bf16 = ml_dtypes.bfloat16

# channel permutation used for wv cols / wp rows / avT partitions:
# p = 64*hp + 32*hh + d  <->  c = 32*(2*hh + hp) + d   (head h = 2*hh + hp)
_PERM = np.array(
    [32 * (2 * ((p % 64) // 32) + p // 64) + p % 32 for p in range(DIM)], dtype=np.int64
)


def _rel_pos_index(ws: int) -> np.ndarray:
    coords = np.stack(np.meshgrid(np.arange(ws), np.arange(ws), indexing="ij"))
    flat = coords.reshape(2, -1)
    rel = flat[:, :, None] - flat[:, None, :]
    rel = rel.transpose(1, 2, 0).astype(np.int64)
    rel[..., 0] += ws - 1
    rel[..., 1] += ws - 1
    rel[..., 0] *= 2 * ws - 1
    return rel.sum(-1)


_BUILT = {}


def _build_program(n_groups=N_G, compile=True, stage=9, bufs=None):
    """Build + compile the single-core Bass program (same program runs SPMD
    on all cores). stage < 9 truncates the pipeline for bisection: that
    stage's intermediate is written to y instead.

    See module docstring for the stage skew; measured 264067 ns/core in
    TimelineSim (baseline kernel: 485740 ns)."""
    bufs = dict(
        dict(sb1=7, sb2=5, qk=2, xp=3, yp=4, att_first=1, hoist=1),
        **(bufs or {}),
    )
    key = (n_groups, compile, stage, tuple(sorted(bufs.items())))
    if key in _BUILT:
        return _BUILT[key]
    n_sc = n_groups * GROUP

    from contextlib import ExitStack

    import concourse.tile as tile
    from concourse import bacc, mybir
    from concourse.masks import make_identity

    f32 = mybir.dt.float32
    bf = mybir.dt.bfloat16
    EXP = mybir.ActivationFunctionType.Exp

    nc = bacc.Bacc("TRN2", target_bir_lowering=False, debug=False, enable_asserts=False)

    x_d = nc.dram_tensor("x", [n_sc * SC_ROWS, DIM], f32, kind="ExternalInput").ap()
    wq_d = nc.dram_tensor("wq_bf", [DIM, DIM], bf, kind="ExternalInput").ap()
    wk_d = nc.dram_tensor("wk_bf", [DIM, DIM], bf, kind="ExternalInput").ap()
    wv_d = nc.dram_tensor("wvp_bf", [DIM, DIM], bf, kind="ExternalInput").ap()
    wp_d = nc.dram_tensor("wpp_bf", [DIM, DIM], bf, kind="ExternalInput").ap()
    # eb[64*hp + m, 256*hb + 64*w + n] = exp(rel_bias[2*hb+hp][n, m]), window-tiled
    eb_d = nc.dram_tensor("expbias_bf", [DIM, 2 * H * N], bf, kind="ExternalInput").ap()
    y_d = nc.dram_tensor("y", [n_sc * SC_ROWS, DIM], f32, kind="ExternalOutput").ap()

    with tile.TileContext(nc) as tc, ExitStack() as ctx:
        consts = ctx.enter_context(tc.tile_pool(name="consts", bufs=1))
        xp = ctx.enter_context(tc.tile_pool(name="xp", bufs=bufs["xp"]))
        sb1 = ctx.enter_context(tc.tile_pool(name="sb1", bufs=bufs["sb1"]))
        sb2 = ctx.enter_context(tc.tile_pool(name="sb2", bufs=bufs["sb2"]))
        yp = ctx.enter_context(tc.tile_pool(name="yp", bufs=bufs["yp"]))

        # PSUM (8 banks of 2KB/partition):
        #   at  [128,1024] f32, manual parity halves        2 banks
        #   qk  [128,512]  f32 x2 bufs                      2 banks
        #   v   [128,256]  f32 x2                           1 bank
        #   av  [128,256]  f32 x2                           1 bank
        #   y   [128,256]  f32 x2                           1 bank
        #   sm: xt_ps bf16 x2 + avT_ps bf16 x1 + s f32 x2   1 bank
        ps_at = ctx.enter_context(tc.tile_pool(name="ps_at", bufs=1, space="PSUM"))
        ps_qk = ctx.enter_context(tc.tile_pool(name="ps_qk", bufs=bufs["qk"], space="PSUM"))
        ps_v = ctx.enter_context(tc.tile_pool(name="ps_v", bufs=1, space="PSUM"))
        ps_av = ctx.enter_context(tc.tile_pool(name="ps_av", bufs=1, space="PSUM"))
        ps_y = ctx.enter_context(tc.tile_pool(name="ps_y", bufs=1, space="PSUM"))
        ps_sm = ctx.enter_context(tc.tile_pool(name="ps_sm", bufs=1, space="PSUM"))

        # constants
        wq = consts.tile([DIM, DIM], bf)
        wk = consts.tile([DIM, DIM], bf)
        wvp = consts.tile([DIM, DIM], bf)
        wpp = consts.tile([DIM, DIM], bf)
        eb = consts.tile([DIM, 2 * H * N], bf)
        nc.sync.dma_start(wq[:], wq_d)
        nc.sync.dma_start(wk[:], wk_d)
        nc.sync.dma_start(wvp[:], wv_d)
        nc.sync.dma_start(wpp[:], wp_d)
        nc.sync.dma_start(eb[:], eb_d)

        identf = consts.tile([DIM, DIM], f32)
        make_identity(nc, identf[:])
        ident = consts.tile([DIM, DIM], bf)
        nc.gpsimd.tensor_copy(ident[:], identf[:])
        ones1 = consts.tile([DIM, 1], bf)
        nc.vector.memset(ones1[:], 1.0)

        live = {}
        cur_ps = {}
        cur_ysb = [None]

        def x_load(g):
            x_g = xp.tile([128, 2 * GROUP, DIM], f32, tag="x_g")
            r0 = g * GROUP * SC_ROWS
            nc.sync.dma_start(
                out=x_g[:],
                in_=x_d[r0 : r0 + GROUP * SC_ROWS, :].rearrange(
                    "(t p) c -> p t c", p=128
                ),
            )
            return x_g

        def st_DC(i):
            """Pool: downcast x -> bf16."""
            sc = live[i]
            so = i % GROUP
            x_bf = sb1.tile([128, 2, DIM], bf, tag="x_bf")
            nc.gpsimd.tensor_copy(x_bf[:], sc["x_g"][:, 2 * so : 2 * so + 2, :])
            sc["x_bf"] = x_bf

        def st_T(i):
            """PE: bf16 transpose -> xt_ps."""
            sc = live[i]
            xt_ps = cur_ps["sm"][:, 128 * (i % 2) : 128 * (i % 2) + 128].bitcast(bf)
            for t in range(2):
                nc.tensor.matmul(
                    xt_ps[:, t * 128 : (t + 1) * 128],
                    sc["x_bf"][:, t, :],
                    ident[:],
                    is_transpose=True,
                    start=True,
                    stop=True,
                )
            sc["xt_ps"] = xt_ps

        def st_XTE(i):
            """DVE: xt evac (bf16 2x)."""
            sc = live[i]
            xt = sb1.tile([128, SC_ROWS], bf, tag="xt")
            nc.vector.tensor_copy(xt[:], sc["xt_ps"][:])
            sc["xt"] = xt

        def st_QKV(i):
            sc = live[i]
            xt = sc["xt"]
            qk_ps = ps_qk.tile([128, 2 * SC_ROWS], f32, tag="qk_ps")
            nc.tensor.matmul(qk_ps[:, 0:SC_ROWS], wq[:], xt[:], start=True, stop=True)
            nc.tensor.matmul(
                qk_ps[:, SC_ROWS : 2 * SC_ROWS], wk[:], xt[:], start=True, stop=True
            )
            v_ps = ps_v.tile([128, 2 * SC_ROWS], f32, tag="v_ps", name="v_ps")[
                :, 256 * (i % 2) : 256 * (i % 2) + SC_ROWS
            ]
            for w in range(SC_W):
                for hp in range(2):
                    nc.tensor.matmul(
                        v_ps[64 * hp : 64 * hp + 64, 64 * w : 64 * (w + 1)],
                        xt[:, 64 * w : 64 * (w + 1)],
                        wvp[:, 64 * hp : 64 * hp + 64],
                        tile_position=(0, 64 * hp),
                        start=True,
                        stop=True,
                    )
            sc["qk_ps"] = qk_ps
            sc["v_ps"] = v_ps

        def st_QKE(i):
            sc = live[i]
            qk = sb1.tile([128, 2 * SC_ROWS], bf, tag="qk")
            nc.scalar.copy(qk[:], sc["qk_ps"][:])
            sc["qk"] = qk

        def st_VE(i):
            sc = live[i]
            vd = sb1.tile([128, SC_ROWS], bf, tag="vd")
            nc.vector.tensor_copy(vd[:], sc["v_ps"][:])
            sc["vd"] = vd

        def st_ATT(i):
            """PE attnT (at_ps halves alternate by sc parity); ACT exp; Pool bias."""
            sc = live[i]
            qk = sc["qk"]
            at_ps = ps_at.tile([128, 1024], f32, tag="at_ps")
            off = 256 * (i % 2)
            for w in range(SC_W):
                for h in range(H):
                    hp, hb = h % 2, h // 2
                    nc.tensor.matmul(
                        at_ps[
                            64 * hp : 64 * hp + 64,
                            512 * hb + off + 64 * w : 512 * hb + off + 64 * (w + 1),
                        ],
                        qk[32 * h : 32 * h + 32, SC_ROWS + 64 * w : SC_ROWS + 64 * (w + 1)],
                        qk[32 * h : 32 * h + 32, 64 * w : 64 * (w + 1)],
                        tile_position=(32 * h, 64 * hp),
                        start=True,
                        stop=True,
                    )
            sc["at_ps"] = at_ps
            sc["at_off"] = off

        def st_EXP(i):
            sc = live[i]
            at_ps, off = sc["at_ps"], sc["at_off"]
            pt = sb2.tile([128, 2 * SC_ROWS], bf, tag="pt")
            nc.scalar.activation(
                pt[:].rearrange("p (b c) -> p b c", b=2),
                at_ps[:].rearrange("p (b c) -> p b c", b=2)[:, :, off : off + SC_ROWS],
                EXP,
            )
            sc["pt"] = pt

        def st_BIAS(i):
            sc = live[i]
            pb = sb2.tile([128, 2 * SC_ROWS], bf, tag="pb")
            if bufs.get("bias_split"):
                nc.vector.tensor_mul(
                    pb[:, 0:SC_ROWS], sc["pt"][:, 0:SC_ROWS], eb[:, 0:SC_ROWS]
                )
                nc.gpsimd.tensor_mul(
                    pb[:, SC_ROWS : 2 * SC_ROWS],
                    sc["pt"][:, SC_ROWS : 2 * SC_ROWS],
                    eb[:, SC_ROWS : 2 * SC_ROWS],
                )
            else:
                eng = nc.vector if bufs.get("bias_dve") else nc.gpsimd
                eng.tensor_mul(pb[:], sc["pt"][:], eb[:, 0 : 2 * SC_ROWS])
            sc["pb"] = pb

        def st_AVS(i):
            """PE: av[n, hd] + sums (P^T stationary)."""
            sc = live[i]
            pb, vd = sc["pb"], sc["vd"]
            av_ps = ps_av.tile([128, 2 * SC_ROWS], f32, tag="av_ps", name="av_ps")[
                :, 256 * (i % 2) : 256 * (i % 2) + SC_ROWS
            ]
            s_ps = cur_ps["sm"][:, 384 + 8 * (i % 2) : 384 + 8 * (i % 2) + 8]
            # sums first: RECIP's dependency completes before the slower av
            # matmuls, pulling the recip->norm->next-AVS chain earlier.
            for w in range(SC_W):
                for h in range(H):
                    hp, hh = h % 2, h // 2
                    nc.tensor.matmul(
                        s_ps[64 * hp : 64 * hp + 64, 2 * w + hh : 2 * w + hh + 1],
                        pb[
                            64 * hp : 64 * hp + 64,
                            256 * hh + 64 * w : 256 * hh + 64 * (w + 1),
                        ],
                        ones1[64 * hp : 64 * hp + 64, :],
                        tile_position=(64 * hp, 64 * hp),
                        start=True,
                        stop=True,
                    )
            for w in range(SC_W):
                for h in range(H):
                    hp, hh = h % 2, h // 2
                    nc.tensor.matmul(
                        av_ps[
                            64 * hp : 64 * hp + 64,
                            64 * w + 32 * hh : 64 * w + 32 * hh + 32,
                        ],
                        pb[
                            64 * hp : 64 * hp + 64,
                            256 * hh + 64 * w : 256 * hh + 64 * (w + 1),
                        ],
                        vd[
                            64 * hp : 64 * hp + 64,
                            64 * w + 32 * hh : 64 * w + 32 * hh + 32,
                        ],
                        tile_position=(64 * hp, 64 * hp),
                        start=True,
                        stop=True,
                    )
            sc["av_ps"] = av_ps
            sc["s_ps"] = s_ps

        def st_RECIP(i):
            sc = live[i]
            rf = sb2.tile([128, 8], f32, tag="rf")
            nc.vector.reciprocal_approx_fast(out=rf[:], in_=sc["s_ps"][:])
            sc["rf"] = rf

        def st_NORM(i):
            sc = live[i]
            av_sb = sb2.tile([128, SC_ROWS], bf, tag="av_sb")
            rf_b = sc["rf"][:].unsqueeze(-1).broadcast_to((128, 8, 32))
            nc.vector.tensor_mul(
                av_sb[:].rearrange("p (k d) -> p k d", k=8),
                sc["av_ps"][:].rearrange("p (k d) -> p k d", k=8),
                rf_b,
            )
            sc["av_sb"] = av_sb

        def st_AVT(i):
            sc = live[i]
            av_sb = sc["av_sb"]
            avT_ps = cur_ps["sm"][:, 256:384].bitcast(bf)
            for w in range(SC_W):
                for hp in range(2):
                    nc.tensor.matmul(
                        avT_ps[64 * hp : 64 * hp + 64, 64 * w : 64 * (w + 1)],
                        av_sb[64 * hp : 64 * hp + 64, 64 * w : 64 * (w + 1)],
                        ident[64 * hp : 64 * hp + 64, 64 * hp : 64 * hp + 64],
                        is_transpose=True,
                        tile_position=(64 * hp, 64 * hp),
                        start=True,
                        stop=True,
                    )
            sc["avT_ps"] = avT_ps

        def st_AVTE(i):
            sc = live[i]
            avt = sb1.tile([128, SC_ROWS], bf, tag="avt")
            nc.vector.tensor_copy(avt[:], sc["avT_ps"][:])
            sc["avt"] = avt

        def st_PROJ(i):
            sc = live[i]
            avt = sc["avt"]
            y_tile = ps_y.tile([128, 4 * DIM], f32, tag="y_ps", name="y_ps")
            y_ps = y_tile[:, 256 * (i % 2) : 256 * (i % 2) + 2 * DIM]
            sc["y_tile"] = y_tile
            for j in range(2):
                nc.tensor.matmul(
                    y_ps[:, 128 * j : 128 * (j + 1)],
                    avt[:, 128 * j : 128 * (j + 1)],
                    wpp[:],
                    start=True,
                    stop=True,
                )
            sc["y_ps"] = y_ps

        def st_YE(i):
            sc = live[i]
            half = i % 2
            if half == 0:
                cur_ysb[0] = yp.tile([128, 4, DIM], f32, tag="y_sb", name="y_sb")
            y_sb = cur_ysb[0]
            nc.scalar.copy(
                y_sb[:, 2 * half : 2 * half + 2, :].rearrange("p a b -> p (a b)"),
                sc["y_ps"][:],
            )
            if half == 1:
                r0 = (i - 1) * SC_ROWS
                nc.sync.dma_start(
                    out=y_d[r0 : r0 + 2 * SC_ROWS, :].rearrange(
                        "(t p) c -> p t c", p=128
                    ),
                    in_=y_sb[:],
                )

        def dump(i, src_ap):
            """Bisect helper (SBUF src only): route [128, 256] to y rows of sc i."""
            half = i % 2
            if half == 0:
                cur_ysb[0] = yp.tile([128, 4, DIM], f32, tag="y_sb", name="y_sb")
            y_sb = cur_ysb[0]
            nc.gpsimd.tensor_copy(
                y_sb[:, 2 * half : 2 * half + 2, :].rearrange("p a b -> p (a b)"),
                src_ap,
            )
            if half == 1:
                r0 = (i - 1) * SC_ROWS
                nc.sync.dma_start(
                    out=y_d[r0 : r0 + 2 * SC_ROWS, :].rearrange(
                        "(t p) c -> p t c", p=128
                    ),
                    in_=y_sb[:],
                )

        def run_iter(it, last):
            cur_ps["sm"] = ps_sm.tile([128, 512], f32, tag="sm", name="sm")
            td = 1 if bufs.get("tail_deep") else 0
            bo = 5 if bufs.get("bias_deep") else 4

            def S(cond, fn, *a):
                if cond:
                    fn(*a)

            if bufs.get("hoist2"):
                # consumers-first: every op whose deps are >=1 iter old is
                # emitted before this iteration's producers.
                S(0 <= it - 9 - td <= last and stage >= 5, st_AVTE, it - 9 - td)
                S(0 <= it - 7 <= last and stage >= 4, st_NORM, it - 7)
                S(0 <= it + 1 <= last, st_XTE, it + 1)
                S(0 <= it - 1 <= last and stage >= 2, st_QKE, it - 1)
                S(0 <= it - 3 <= last and stage >= 3, st_EXP, it - 3)
                S(0 <= it - 11 - td <= last and stage >= 5, st_YE, it - 11 - td)
                S(0 <= it - bo <= last and stage >= 3, st_BIAS, it - bo)
                if it + 7 <= last + 4 and (it + 7) % GROUP == 0:
                    g = (it + 7) // GROUP
                    if 0 < g < n_groups:
                        live_g = x_load(g)
                        for k in range(GROUP):
                            live.setdefault(g * GROUP + k, {})["x_g"] = live_g
                S(0 <= it + 3 <= last, st_DC, it + 3)
                S(0 <= it - 2 <= last and stage >= 3, st_ATT, it - 2)
                S(0 <= it + 2 <= last, st_T, it + 2)
                S(0 <= it <= last and stage >= 2, st_QKV, it)
                S(0 <= it - 6 <= last and stage >= 4, st_AVS, it - 6)
                S(0 <= it - 8 - td <= last and stage >= 5, st_AVT, it - 8 - td)
                S(0 <= it - 10 - td <= last and stage >= 5, st_PROJ, it - 10 - td)
                S(0 <= it - 1 <= last and stage >= 2, st_VE, it - 1)
                S(0 <= it - 6 <= last and stage >= 4, st_RECIP, it - 6)
                if stage < 2 and 0 <= it + 1 <= last:
                    dump(it + 1, live[it + 1]["xt"][:])
                if stage == 2 and 0 <= it - 1 <= last:
                    dump(it - 1, live[it - 1]["qk"][:, 0:SC_ROWS])
                if stage == 3 and 0 <= it - bo <= last:
                    dump(it - bo, live[it - bo]["pb"][:, 0:SC_ROWS])
                if stage == 4 and 0 <= it - 7 <= last:
                    dump(it - 7, live[it - 7]["av_sb"][:])
                if 0 <= it - 12 - td <= last and stage >= 5:
                    live.pop(it - 12 - td)
                return

            if bufs.get("hoist"):
                td0 = 1 if bufs.get("tail_deep") else 0
                if 0 <= it - 9 - td0 <= last and stage >= 5:
                    st_AVTE(it - 9 - td0)
                if 0 <= it - 7 <= last and stage >= 4:
                    st_NORM(it - 7)
            if bufs.get("att_first") and 0 <= it - 2 <= last and stage >= 3:
                st_ATT(it - 2)
            if it + 7 <= last + 4 and (it + 7) % GROUP == 0:
                g = (it + 7) // GROUP
                if 0 < g < n_groups:
                    live_g = x_load(g)
                    for k in range(GROUP):
                        live.setdefault(g * GROUP + k, {})["x_g"] = live_g
            if 0 <= it + 3 <= last:
                st_DC(it + 3)
            if 0 <= it + 2 <= last:
                st_T(it + 2)
            if 0 <= it + 1 <= last:
                st_XTE(it + 1)
                if stage < 2:
                    dump(it + 1, live[it + 1]["xt"][:])
            if 0 <= it <= last and stage >= 2:
                st_QKV(it)
            if 0 <= it - 1 <= last and stage >= 2:
                st_QKE(it - 1)
                st_VE(it - 1)
                if stage < 3:
                    dump(it - 1, live[it - 1]["qk"][:, 0:SC_ROWS])
            if not bufs.get("att_first") and 0 <= it - 2 <= last and stage >= 3:
                st_ATT(it - 2)
            if 0 <= it - 3 <= last and stage >= 3:
                st_EXP(it - 3)
            if 0 <= it - bo <= last and stage >= 3:
                st_BIAS(it - bo)
                if stage < 4:
                    dump(it - bo, live[it - bo]["pb"][:, 0:SC_ROWS])
            if 0 <= it - 6 <= last and stage >= 4:
                st_AVS(it - 6)
            if 0 <= it - 7 <= last and stage >= 4:
                if not bufs.get("hoist"):
                    st_NORM(it - 7)
                if stage < 5:
                    dump(it - 7, live[it - 7]["av_sb"][:])
            if not bufs.get("hoist") and 0 <= it - 9 - td <= last and stage >= 5:
                st_AVTE(it - 9 - td)
            if 0 <= it - 8 - td <= last and stage >= 5:
                st_AVT(it - 8 - td)
            if 0 <= it - 10 - td <= last and stage >= 5:
                st_PROJ(it - 10 - td)
            if 0 <= it - 11 - td <= last and stage >= 5:
                st_YE(it - 11 - td)
                live.pop(it - 11 - td)
            if not bufs.get("recip7") and 0 <= it - 6 <= last and stage >= 4:
                st_RECIP(it - 6)

        g0 = x_load(0)
        for k in range(GROUP):
            live.setdefault(k, {})["x_g"] = g0
        for it in range(-3, n_sc + 12 + (1 if bufs.get("tail_deep") else 0)):
            run_iter(it, n_sc - 1)

    if compile:
        nc.compile()
    _BUILT[key] = nc
    return nc


def _host_prep(w_qkv, w_proj, bias_table):
    """Precompute replicated small tensors (channel-permuted for the kernel)."""
    scale = HD**-0.5
    wq = (w_qkv[:, :DIM] * scale).astype(bf16)
    wk = w_qkv[:, DIM : 2 * DIM].astype(bf16)
    wv = w_qkv[:, 2 * DIM :]
    wvp = np.ascontiguousarray(wv[:, _PERM]).astype(bf16)
    wpp = np.ascontiguousarray(w_proj[_PERM, :]).astype(bf16)

    rel = _rel_pos_index(WS)  # [N, N]
    rel_bias = bias_table[rel.reshape(-1)].reshape(N, N, H).transpose(2, 0, 1)  # [h,n,m]
    ebv = np.exp(rel_bias).astype(np.float32)  # [h, n, m]
    # eb[64*hp + m, 256*hb + 64*w + n] = ebv[2*hb + hp][n, m]
    eb = np.zeros((DIM, 512), np.float32)
    for hb in range(2):
        for hp in range(2):
            h = 2 * hb + hp
            blk = ebv[h].T  # [m, n]
            for w in range(SC_W):
                eb[
                    64 * hp : 64 * hp + 64, 256 * hb + 64 * w : 256 * hb + 64 * (w + 1)
                ] = blk
    eb = eb.astype(bf16)
    return wq, wk, wvp, wpp, eb


def run(x, w_qkv, w_proj, b_proj, bias_table, trace=False, **trace_kwargs):
    """Run on 8 NeuronCores. Returns (y, BassKernelResults)."""
    from concourse import bass_utils

    x = np.asarray(x, dtype=np.float32)
    w_qkv = np.asarray(w_qkv, dtype=np.float32)
    w_proj = np.asarray(w_proj, dtype=np.float32)
    b_proj = np.asarray(b_proj, dtype=np.float32)
    bias_table = np.asarray(bias_table, dtype=np.float32)

    wq, wk, wvp, wpp, eb = _host_prep(w_qkv, w_proj, bias_table)
    nc = _build_program()

    xs = x.reshape(B * N, DIM)
    in_maps = []
    for c in range(NCORES):
        in_maps.append(
            {
                "x": np.ascontiguousarray(xs[c * ROWS : (c + 1) * ROWS]),
                "wq_bf": wq,
                "wk_bf": wk,
                "wvp_bf": wvp,
                "wpp_bf": wpp,
                "expbias_bf": eb,
            }
        )

    res = bass_utils.run_bass_kernel_spmd(
        nc, in_maps, core_ids=list(range(NCORES)), trace=trace, **trace_kwargs
    )
    y = np.concatenate([res.results[c]["y"] for c in range(NCORES)], axis=0)
    y = y + b_proj[None, :]
    return y.reshape(B, N, DIM), res


def kernel(x, w_qkv, w_proj, b_proj, bias_table):
    y, _ = run(x, w_qkv, w_proj, b_proj, bias_table)
    return y


if __name__ == "__main__":
    sys.path.insert(0, "/root/problem")
    import reference

    inputs = {k: np.asarray(v) for k, v in reference.setup_inputs().items()}
    out = kernel(**inputs)
    exp = np.asarray(reference.reference(**inputs))
    err = np.abs(out - exp)
    print("abs max err:", err.max(), "scale-rel:", err.max() / np.abs(exp).max())


# revision 48
# speedup vs baseline: 1.0102x; 1.0005x over previous
"""Trainium2 Bass kernel for windowed multi-head attention (Swin-style).

Problem: B=4096 windows x N=64 tokens x C=128 channels, H=4 heads, hd=32.
  qkv = x @ w_qkv ; attn = softmax(q k^T / sqrt(hd) + rel_bias) ; out = (attn v) @ w_proj + b

Sharding: data-parallel over windows, 512 windows per core on 8 cores.

Dataflow (per superchunk of 4 windows = 256 tokens; matmul operands bf16):
  x downcast on GPSIMD; xT via PE bf16 transpose; qT/kT weight-stationary (512 cols)
  v[m, c-perm] head-parity packed (512 cols, no duplication)
  attnT[m, n] per (window, head) (1024 cols)
  P = exp(attnT) (ACT) * exp(bias) (DVE bf16)
  av[n, hd] with P^T stationary (512 cols) + softmax sums as 1-col matmuls (16)
  av_sb = av * recip(s) broadcast (DVE, fused evacuation)
  avT via 8 PE [64,64] transposes (512 cols); y = avT @ wp_perm (256 cols)
  b_proj added on HOST after gather.

The per-superchunk stages are emitted SOFTWARE-PIPELINED with a deep skew;
at iteration i the emitted stage instances are
  ATT(i-2) DC(i+3) T(i+2) XTE(i+1) QKV(i) QKE/VE(i-1) EXP(i-3) BIAS(i-4)
  AVS(i-6) NORM(i-7) AVTE(i-9) AVT(i-8) PROJ(i-10) YE(i-11) RECIP(i-6)
so every cross-engine dependency (except the deliberate tail RECIP) is at
least one iteration old and no engine head-of-line blocks. Engine split:
PE matmuls ~1.50us/iter, ACT {qk-evac, exp, y-evac} ~1.63us, DVE {xt/v/avT
evacs, recip, norm} ~1.43us, Pool (GPSIMD, no PSUM access!) {x downcast,
bias mul} ~1.56us, DMA ~0.75us. x is group-prefetched ~1.5 groups ahead.

PSUM-bank drain rule respected: no two in-flight matmuls with different
tile_position row-groups and the same column-group share a PSUM bank.
"""

import sys

sys.path.insert(0, "/opt/trn_rl_repo")

import numpy as np
import ml_dtypes

WS = 8
H = 4
DIM = 128
N = WS * WS  # 64 tokens per window
HD = DIM // H  # 32
B = 4096
NCORES = 8
BC = B // NCORES  # 512 windows per core
ROWS = BC * N  # 32768 rows per core

SC_W = 4  # windows per superchunk
SC_ROWS = SC_W * N  # 256
N_SC = BC // SC_W  # 128 superchunks
GROUP = 4  # superchunks per input DMA group
N_G = N_SC // GROUP  # 32 groups

bf16 = ml_dtypes.bfloat16

# channel permutation used for wv cols / wp rows / avT partitions:
# p = 64*hp + 32*hh + d  <->  c = 32*(2*hh + hp) + d   (head h = 2*hh + hp)
_PERM = np.array(
    [32 * (2 * ((p % 64) // 32) + p // 64) + p % 32 for p in range(DIM)], dtype=np.int64
)


def _rel_pos_index(ws: int) -> np.ndarray:
    coords = np.stack(np.meshgrid(np.arange(ws), np.arange(ws), indexing="ij"))
    flat = coords.reshape(2, -1)
    rel = flat[:, :, None] - flat[:, None, :]
    rel = rel.transpose(1, 2, 0).astype(np.int64)
    rel[..., 0] += ws - 1
    rel[..., 1] += ws - 1
    rel[..., 0] *= 2 * ws - 1
    return rel.sum(-1)


_BUILT = {}


def _build_program(n_groups=N_G, compile=True, stage=9, bufs=None):
    """Build + compile the single-core Bass program (same program runs SPMD
    on all cores). stage < 9 truncates the pipeline for bisection: that
    stage's intermediate is written to y instead.

    See module docstring for the stage skew; measured 264067 ns/core in
    TimelineSim (baseline kernel: 485740 ns)."""
    bufs = dict(
        dict(sb1=7, sb2=5, qk=2, xp=4, yp=5, att_first=1, hoist=1),
        **(bufs or {}),
    )
    key = (n_groups, compile, stage, tuple(sorted(bufs.items())))
    if key in _BUILT:
        return _BUILT[key]
    n_sc = n_groups * GROUP

    from contextlib import ExitStack

    import concourse.tile as tile
    from concourse import bacc, mybir
    from concourse.masks import make_identity

    f32 = mybir.dt.float32
    bf = mybir.dt.bfloat16
    EXP = mybir.ActivationFunctionType.Exp

    nc = bacc.Bacc("TRN2", target_bir_lowering=False, debug=False, enable_asserts=False)

    x_d = nc.dram_tensor("x", [n_sc * SC_ROWS, DIM], f32, kind="ExternalInput").ap()
    wq_d = nc.dram_tensor("wq_bf", [DIM, DIM], bf, kind="ExternalInput").ap()
    wk_d = nc.dram_tensor("wk_bf", [DIM, DIM], bf, kind="ExternalInput").ap()
    wv_d = nc.dram_tensor("wvp_bf", [DIM, DIM], bf, kind="ExternalInput").ap()
    wp_d = nc.dram_tensor("wpp_bf", [DIM, DIM], bf, kind="ExternalInput").ap()
    # eb[64*hp + m, 256*hb + 64*w + n] = exp(rel_bias[2*hb+hp][n, m]), window-tiled
    eb_d = nc.dram_tensor("expbias_bf", [DIM, 2 * H * N], bf, kind="ExternalInput").ap()
    y_d = nc.dram_tensor("y", [n_sc * SC_ROWS, DIM], f32, kind="ExternalOutput").ap()

    with tile.TileContext(nc) as tc, ExitStack() as ctx:
        consts = ctx.enter_context(tc.tile_pool(name="consts", bufs=1))
        xp = ctx.enter_context(tc.tile_pool(name="xp", bufs=bufs["xp"]))
        sb1 = ctx.enter_context(tc.tile_pool(name="sb1", bufs=bufs["sb1"]))
        sb2 = ctx.enter_context(tc.tile_pool(name="sb2", bufs=bufs["sb2"]))
        yp = ctx.enter_context(tc.tile_pool(name="yp", bufs=bufs["yp"]))

        # PSUM (8 banks of 2KB/partition):
        #   at  [128,1024] f32, manual parity halves        2 banks
        #   qk  [128,512]  f32 x2 bufs                      2 banks
        #   v   [128,256]  f32 x2                           1 bank
        #   av  [128,256]  f32 x2                           1 bank
        #   y   [128,256]  f32 x2                           1 bank
        #   sm: xt_ps bf16 x2 + avT_ps bf16 x1 + s f32 x2   1 bank
        ps_at = ctx.enter_context(tc.tile_pool(name="ps_at", bufs=1, space="PSUM"))
        ps_qk = ctx.enter_context(tc.tile_pool(name="ps_qk", bufs=bufs["qk"], space="PSUM"))
        ps_v = ctx.enter_context(tc.tile_pool(name="ps_v", bufs=1, space="PSUM"))
        ps_av = ctx.enter_context(tc.tile_pool(name="ps_av", bufs=1, space="PSUM"))
        ps_y = ctx.enter_context(tc.tile_pool(name="ps_y", bufs=1, space="PSUM"))
        ps_sm = ctx.enter_context(tc.tile_pool(name="ps_sm", bufs=1, space="PSUM"))

        # constants
        wq = consts.tile([DIM, DIM], bf)
        wk = consts.tile([DIM, DIM], bf)
        wvp = consts.tile([DIM, DIM], bf)
        wpp = consts.tile([DIM, DIM], bf)
        eb = consts.tile([DIM, 2 * H * N], bf)
        nc.sync.dma_start(wq[:], wq_d)
        nc.sync.dma_start(wk[:], wk_d)
        nc.sync.dma_start(wvp[:], wv_d)
        nc.sync.dma_start(wpp[:], wp_d)
        nc.sync.dma_start(eb[:], eb_d)

        identf = consts.tile([DIM, DIM], f32)
        make_identity(nc, identf[:])
        ident = consts.tile([DIM, DIM], bf)
        nc.gpsimd.tensor_copy(ident[:], identf[:])
        ones1 = consts.tile([DIM, 1], bf)
        nc.vector.memset(ones1[:], 1.0)

        live = {}
        cur_ps = {}
        cur_ysb = [None]

        def x_load(g):
            x_g = xp.tile([128, 2 * GROUP, DIM], f32, tag="x_g")
            r0 = g * GROUP * SC_ROWS
            nc.sync.dma_start(
                out=x_g[:],
                in_=x_d[r0 : r0 + GROUP * SC_ROWS, :].rearrange(
                    "(t p) c -> p t c", p=128
                ),
            )
            return x_g

        def st_DC(i):
            """Pool: downcast x -> bf16."""
            sc = live[i]
            so = i % GROUP
            x_bf = sb1.tile([128, 2, DIM], bf, tag="x_bf")
            nc.gpsimd.tensor_copy(x_bf[:], sc["x_g"][:, 2 * so : 2 * so + 2, :])
            sc["x_bf"] = x_bf

        def st_T(i):
            """PE: bf16 transpose -> xt_ps."""
            sc = live[i]
            xt_ps = cur_ps["sm"][:, 128 * (i % 2) : 128 * (i % 2) + 128].bitcast(bf)
            for t in range(2):
                nc.tensor.matmul(
                    xt_ps[:, t * 128 : (t + 1) * 128],
                    sc["x_bf"][:, t, :],
                    ident[:],
                    is_transpose=True,
                    start=True,
                    stop=True,
                )
            sc["xt_ps"] = xt_ps

        def st_XTE(i):
            """DVE: xt evac (bf16 2x)."""
            sc = live[i]
            xt = sb1.tile([128, SC_ROWS], bf, tag="xt")
            nc.vector.tensor_copy(xt[:], sc["xt_ps"][:])
            sc["xt"] = xt

        def st_QKV(i):
            sc = live[i]
            xt = sc["xt"]
            qk_ps = ps_qk.tile([128, 2 * SC_ROWS], f32, tag="qk_ps")
            nc.tensor.matmul(qk_ps[:, 0:SC_ROWS], wq[:], xt[:], start=True, stop=True)
            nc.tensor.matmul(
                qk_ps[:, SC_ROWS : 2 * SC_ROWS], wk[:], xt[:], start=True, stop=True
            )
            v_ps = ps_v.tile([128, 2 * SC_ROWS], f32, tag="v_ps", name="v_ps")[
                :, 256 * (i % 2) : 256 * (i % 2) + SC_ROWS
            ]
            for w in range(SC_W):
                for hp in range(2):
                    nc.tensor.matmul(
                        v_ps[64 * hp : 64 * hp + 64, 64 * w : 64 * (w + 1)],
                        xt[:, 64 * w : 64 * (w + 1)],
                        wvp[:, 64 * hp : 64 * hp + 64],
                        tile_position=(0, 64 * hp),
                        start=True,
                        stop=True,
                    )
            sc["qk_ps"] = qk_ps
            sc["v_ps"] = v_ps

        def st_QKE(i):
            sc = live[i]
            qk = sb1.tile([128, 2 * SC_ROWS], bf, tag="qk")
            nc.scalar.copy(qk[:], sc["qk_ps"][:])
            sc["qk"] = qk

        def st_VE(i):
            sc = live[i]
            vd = sb1.tile([128, SC_ROWS], bf, tag="vd")
            nc.vector.tensor_copy(vd[:], sc["v_ps"][:])
            sc["vd"] = vd

        def st_ATT(i):
            """PE attnT (at_ps halves alternate by sc parity); ACT exp; Pool bias."""
            sc = live[i]
            qk = sc["qk"]
            at_ps = ps_at.tile([128, 1024], f32, tag="at_ps")
            off = 256 * (i % 2)
            for w in range(SC_W):
                for h in range(H):
                    hp, hb = h % 2, h // 2
                    nc.tensor.matmul(
                        at_ps[
                            64 * hp : 64 * hp + 64,
                            512 * hb + off + 64 * w : 512 * hb + off + 64 * (w + 1),
                        ],
                        qk[32 * h : 32 * h + 32, SC_ROWS + 64 * w : SC_ROWS + 64 * (w + 1)],
                        qk[32 * h : 32 * h + 32, 64 * w : 64 * (w + 1)],
                        tile_position=(32 * h, 64 * hp),
                        start=True,
                        stop=True,
                    )
            sc["at_ps"] = at_ps
            sc["at_off"] = off

        def st_EXP(i):
            sc = live[i]
            at_ps, off = sc["at_ps"], sc["at_off"]
            pt = sb2.tile([128, 2 * SC_ROWS], bf, tag="pt")
            nc.scalar.activation(
                pt[:].rearrange("p (b c) -> p b c", b=2),
                at_ps[:].rearrange("p (b c) -> p b c", b=2)[:, :, off : off + SC_ROWS],
                EXP,
            )
            sc["pt"] = pt

        def st_BIAS(i):
            sc = live[i]
            pb = sb2.tile([128, 2 * SC_ROWS], bf, tag="pb")
            if bufs.get("bias_split"):
                nc.vector.tensor_mul(
                    pb[:, 0:SC_ROWS], sc["pt"][:, 0:SC_ROWS], eb[:, 0:SC_ROWS]
                )
                nc.gpsimd.tensor_mul(
                    pb[:, SC_ROWS : 2 * SC_ROWS],
                    sc["pt"][:, SC_ROWS : 2 * SC_ROWS],
                    eb[:, SC_ROWS : 2 * SC_ROWS],
                )
            else:
                eng = nc.vector if bufs.get("bias_dve") else nc.gpsimd
                eng.tensor_mul(pb[:], sc["pt"][:], eb[:, 0 : 2 * SC_ROWS])
            sc["pb"] = pb

        def st_AVS(i):
            """PE: av[n, hd] + sums (P^T stationary)."""
            sc = live[i]
            pb, vd = sc["pb"], sc["vd"]
            av_ps = ps_av.tile([128, 2 * SC_ROWS], f32, tag="av_ps", name="av_ps")[
                :, 256 * (i % 2) : 256 * (i % 2) + SC_ROWS
            ]
            s_ps = cur_ps["sm"][:, 384 + 8 * (i % 2) : 384 + 8 * (i % 2) + 8]
            # sums first: RECIP's dependency completes before the slower av
            # matmuls, pulling the recip->norm->next-AVS chain earlier.
            for w in range(SC_W):
                for h in range(H):
                    hp, hh = h % 2, h // 2
                    nc.tensor.matmul(
                        s_ps[64 * hp : 64 * hp + 64, 2 * w + hh : 2 * w + hh + 1],
                        pb[
                            64 * hp : 64 * hp + 64,
                            256 * hh + 64 * w : 256 * hh + 64 * (w + 1),
                        ],
                        ones1[64 * hp : 64 * hp + 64, :],
                        tile_position=(64 * hp, 64 * hp),
                        start=True,
                        stop=True,
                    )
            for w in range(SC_W):
                for h in range(H):
                    hp, hh = h % 2, h // 2
                    nc.tensor.matmul(
                        av_ps[
                            64 * hp : 64 * hp + 64,
                            64 * w + 32 * hh : 64 * w + 32 * hh + 32,
                        ],
                        pb[
                            64 * hp : 64 * hp + 64,
                            256 * hh + 64 * w : 256 * hh + 64 * (w + 1),
                        ],
                        vd[
                            64 * hp : 64 * hp + 64,
                            64 * w + 32 * hh : 64 * w + 32 * hh + 32,
                        ],
                        tile_position=(64 * hp, 64 * hp),
                        start=True,
                        stop=True,
                    )
            sc["av_ps"] = av_ps
            sc["s_ps"] = s_ps

        def st_RECIP(i):
            sc = live[i]
            rf = sb2.tile([128, 8], f32, tag="rf")
            nc.vector.reciprocal_approx_fast(out=rf[:], in_=sc["s_ps"][:])
            sc["rf"] = rf

        def st_NORM(i):
            sc = live[i]
            av_sb = sb2.tile([128, SC_ROWS], bf, tag="av_sb")
            rf_b = sc["rf"][:].unsqueeze(-1).broadcast_to((128, 8, 32))
            nc.vector.tensor_mul(
                av_sb[:].rearrange("p (k d) -> p k d", k=8),
                sc["av_ps"][:].rearrange("p (k d) -> p k d", k=8),
                rf_b,
            )
            sc["av_sb"] = av_sb

        def st_AVT(i):
            sc = live[i]
            av_sb = sc["av_sb"]
            avT_ps = cur_ps["sm"][:, 256:384].bitcast(bf)
            for w in range(SC_W):
                for hp in range(2):
                    nc.tensor.matmul(
                        avT_ps[64 * hp : 64 * hp + 64, 64 * w : 64 * (w + 1)],
                        av_sb[64 * hp : 64 * hp + 64, 64 * w : 64 * (w + 1)],
                        ident[64 * hp : 64 * hp + 64, 64 * hp : 64 * hp + 64],
                        is_transpose=True,
                        tile_position=(64 * hp, 64 * hp),
                        start=True,
                        stop=True,
                    )
            sc["avT_ps"] = avT_ps

        def st_AVTE(i):
            sc = live[i]
            avt = sb1.tile([128, SC_ROWS], bf, tag="avt")
            nc.vector.tensor_copy(avt[:], sc["avT_ps"][:])
            sc["avt"] = avt

        def st_PROJ(i):
            sc = live[i]
            avt = sc["avt"]
            y_tile = ps_y.tile([128, 4 * DIM], f32, tag="y_ps", name="y_ps")
            y_ps = y_tile[:, 256 * (i % 2) : 256 * (i % 2) + 2 * DIM]
            sc["y_tile"] = y_tile
            for j in range(2):
                nc.tensor.matmul(
                    y_ps[:, 128 * j : 128 * (j + 1)],
                    avt[:, 128 * j : 128 * (j + 1)],
                    wpp[:],
                    start=True,
                    stop=True,
                )
            sc["y_ps"] = y_ps

        def st_YE(i):
            sc = live[i]
            half = i % 2
            if half == 0:
                cur_ysb[0] = yp.tile([128, 4, DIM], f32, tag="y_sb", name="y_sb")
            y_sb = cur_ysb[0]
            nc.scalar.copy(
                y_sb[:, 2 * half : 2 * half + 2, :].rearrange("p a b -> p (a b)"),
                sc["y_ps"][:],
            )
            if half == 1:
                r0 = (i - 1) * SC_ROWS
                nc.sync.dma_start(
                    out=y_d[r0 : r0 + 2 * SC_ROWS, :].rearrange(
                        "(t p) c -> p t c", p=128
                    ),
                    in_=y_sb[:],
                )

        def dump(i, src_ap):
            """Bisect helper (SBUF src only): route [128, 256] to y rows of sc i."""
            half = i % 2
            if half == 0:
                cur_ysb[0] = yp.tile([128, 4, DIM], f32, tag="y_sb", name="y_sb")
            y_sb = cur_ysb[0]
            nc.gpsimd.tensor_copy(
                y_sb[:, 2 * half : 2 * half + 2, :].rearrange("p a b -> p (a b)"),
                src_ap,
            )
            if half == 1:
                r0 = (i - 1) * SC_ROWS
                nc.sync.dma_start(
                    out=y_d[r0 : r0 + 2 * SC_ROWS, :].rearrange(
                        "(t p) c -> p t c", p=128
                    ),
                    in_=y_sb[:],
                )

        def run_iter(it, last):
            cur_ps["sm"] = ps_sm.tile([128, 512], f32, tag="sm", name="sm")
            td = 1 if bufs.get("tail_deep") else 0
            bo = 5 if bufs.get("bias_deep") else 4

            def S(cond, fn, *a):
                if cond:
                    fn(*a)

            if bufs.get("hoist2"):
                # consumers-first: every op whose deps are >=1 iter old is
                # emitted before this iteration's producers.
                S(0 <= it - 9 - td <= last and stage >= 5, st_AVTE, it - 9 - td)
                S(0 <= it - 7 <= last and stage >= 4, st_NORM, it - 7)
                S(0 <= it + 1 <= last, st_XTE, it + 1)
                S(0 <= it - 1 <= last and stage >= 2, st_QKE, it - 1)
                S(0 <= it - 3 <= last and stage >= 3, st_EXP, it - 3)
                S(0 <= it - 11 - td <= last and stage >= 5, st_YE, it - 11 - td)
                S(0 <= it - bo <= last and stage >= 3, st_BIAS, it - bo)
                if it + 7 <= last + 4 and (it + 7) % GROUP == 0:
                    g = (it + 7) // GROUP
                    if 0 < g < n_groups:
                        live_g = x_load(g)
                        for k in range(GROUP):
                            live.setdefault(g * GROUP + k, {})["x_g"] = live_g
                S(0 <= it + 3 <= last, st_DC, it + 3)
                S(0 <= it - 2 <= last and stage >= 3, st_ATT, it - 2)
                S(0 <= it + 2 <= last, st_T, it + 2)
                S(0 <= it <= last and stage >= 2, st_QKV, it)
                S(0 <= it - 6 <= last and stage >= 4, st_AVS, it - 6)
                S(0 <= it - 8 - td <= last and stage >= 5, st_AVT, it - 8 - td)
                S(0 <= it - 10 - td <= last and stage >= 5, st_PROJ, it - 10 - td)
                S(0 <= it - 1 <= last and stage >= 2, st_VE, it - 1)
                S(0 <= it - 6 <= last and stage >= 4, st_RECIP, it - 6)
                if stage < 2 and 0 <= it + 1 <= last:
                    dump(it + 1, live[it + 1]["xt"][:])
                if stage == 2 and 0 <= it - 1 <= last:
                    dump(it - 1, live[it - 1]["qk"][:, 0:SC_ROWS])
                if stage == 3 and 0 <= it - bo <= last:
                    dump(it - bo, live[it - bo]["pb"][:, 0:SC_ROWS])
                if stage == 4 and 0 <= it - 7 <= last:
                    dump(it - 7, live[it - 7]["av_sb"][:])
                if 0 <= it - 12 - td <= last and stage >= 5:
                    live.pop(it - 12 - td)
                return

            if bufs.get("hoist"):
                td0 = 1 if bufs.get("tail_deep") else 0
                if 0 <= it - 9 - td0 <= last and stage >= 5:
                    st_AVTE(it - 9 - td0)
                if 0 <= it - 7 <= last and stage >= 4:
                    st_NORM(it - 7)
            if bufs.get("att_first") and 0 <= it - 2 <= last and stage >= 3:
                st_ATT(it - 2)
            if it + 7 <= last + 4 and (it + 7) % GROUP == 0:
                g = (it + 7) // GROUP
                if 0 < g < n_groups:
                    live_g = x_load(g)
                    for k in range(GROUP):
                        live.setdefault(g * GROUP + k, {})["x_g"] = live_g
            if 0 <= it + 3 <= last:
                st_DC(it + 3)
            if 0 <= it + 2 <= last:
                st_T(it + 2)
            if 0 <= it + 1 <= last:
                st_XTE(it + 1)
                if stage < 2:
                    dump(it + 1, live[it + 1]["xt"][:])
            if 0 <= it <= last and stage >= 2:
                st_QKV(it)
            if 0 <= it - 1 <= last and stage >= 2:
                st_QKE(it - 1)
                st_VE(it - 1)
                if stage < 3:
                    dump(it - 1, live[it - 1]["qk"][:, 0:SC_ROWS])
            if not bufs.get("att_first") and 0 <= it - 2 <= last and stage >= 3:
                st_ATT(it - 2)
            if 0 <= it - 3 <= last and stage >= 3:
                st_EXP(it - 3)
            if 0 <= it - bo <= last and stage >= 3:
                st_BIAS(it - bo)
                if stage < 4:
                    dump(it - bo, live[it - bo]["pb"][:, 0:SC_ROWS])
            if 0 <= it - 6 <= last and stage >= 4:
                st_AVS(it - 6)
            if 0 <= it - 7 <= last and stage >= 4:
                if not bufs.get("hoist"):
                    st_NORM(it - 7)
                if stage < 5:
                    dump(it - 7, live[it - 7]["av_sb"][:])
            if not bufs.get("hoist") and 0 <= it - 9 - td <= last and stage >= 5:
                st_AVTE(it - 9 - td)
            if 0 <= it - 8 - td <= last and stage >= 5:
                st_AVT(it - 8 - td)
            if 0 <= it - 10 - td <= last and stage >= 5:
                st_PROJ(it - 10 - td)
            if 0 <= it - 11 - td <= last and stage >= 5:
                st_YE(it - 11 - td)
                live.pop(it - 11 - td)
            if not bufs.get("recip7") and 0 <= it - 6 <= last and stage >= 4:
                st_RECIP(it - 6)

        g0 = x_load(0)
        for k in range(GROUP):
            live.setdefault(k, {})["x_g"] = g0
        for it in range(-3, n_sc + 12 + (1 if bufs.get("tail_deep") else 0)):
            run_iter(it, n_sc - 1)

    if compile:
        nc.compile()
    _BUILT[key] = nc
    return nc


def _host_prep(w_qkv, w_proj, bias_table):
    """Precompute replicated small tensors (channel-permuted for the kernel)."""
    scale = HD**-0.5
    wq = (w_qkv[:, :DIM] * scale).astype(bf16)
    wk = w_qkv[:, DIM : 2 * DIM].astype(bf16)
    wv = w_qkv[:, 2 * DIM :]
    wvp = np.ascontiguousarray(wv[:, _PERM]).astype(bf16)
    wpp = np.ascontiguousarray(w_proj[_PERM, :]).astype(bf16)

    rel = _rel_pos_index(WS)  # [N, N]
    rel_bias = bias_table[rel.reshape(-1)].reshape(N, N, H).transpose(2, 0, 1)  # [h,n,m]
    ebv = np.exp(rel_bias).astype(np.float32)  # [h, n, m]
    # eb[64*hp + m, 256*hb + 64*w + n] = ebv[2*hb + hp][n, m]
    eb = np.zeros((DIM, 512), np.float32)
    for hb in range(2):
        for hp in range(2):
            h = 2 * hb + hp
            blk = ebv[h].T  # [m, n]
            for w in range(SC_W):
                eb[
                    64 * hp : 64 * hp + 64, 256 * hb + 64 * w : 256 * hb + 64 * (w + 1)
                ] = blk
    eb = eb.astype(bf16)
    return wq, wk, wvp, wpp, eb


def run(x, w_qkv, w_proj, b_proj, bias_table, trace=False, **trace_kwargs):
    """Run on 8 NeuronCores. Returns (y, BassKernelResults)."""
    from concourse import bass_utils

    x = np.asarray(x, dtype=np.float32)
    w_qkv = np.asarray(w_qkv, dtype=np.float32)
    w_proj = np.asarray(w_proj, dtype=np.float32)
    b_proj = np.asarray(b_proj, dtype=np.float32)
    bias_table = np.asarray(bias_table, dtype=np.float32)

    wq, wk, wvp, wpp, eb = _host_prep(w_qkv, w_proj, bias_table)
    nc = _build_program()

    xs = x.reshape(B * N, DIM)
    in_maps = []
    for c in range(NCORES):
        in_maps.append(
            {
                "x": np.ascontiguousarray(xs[c * ROWS : (c + 1) * ROWS]),
                "wq_bf": wq,
                "wk_bf": wk,
                "wvp_bf": wvp,
                "wpp_bf": wpp,
                "expbias_bf": eb,
            }
        )

    res = bass_utils.run_bass_kernel_spmd(
        nc, in_maps, core_ids=list(range(NCORES)), trace=trace, **trace_kwargs
    )
    y = np.concatenate([res.results[c]["y"] for c in range(NCORES)], axis=0)
    y = y + b_proj[None, :]
    return y.reshape(B, N, DIM), res


def kernel(x, w_qkv, w_proj, b_proj, bias_table):
    y, _ = run(x, w_qkv, w_proj, b_proj, bias_table)
    return y


if __name__ == "__main__":
    sys.path.insert(0, "/root/problem")
    import reference

    inputs = {k: np.asarray(v) for k, v in reference.setup_inputs().items()}
    out = kernel(**inputs)
    exp = np.asarray(reference.reference(**inputs))
    err = np.abs(out - exp)
    print("abs max err:", err.max(), "scale-rel:", err.max() / np.abs(exp).max())


# revision 50
# speedup vs baseline: 1.0105x; 1.0003x over previous
"""Trainium2 Bass kernel for windowed multi-head attention (Swin-style).

Problem: B=4096 windows x N=64 tokens x C=128 channels, H=4 heads, hd=32.
  qkv = x @ w_qkv ; attn = softmax(q k^T / sqrt(hd) + rel_bias) ; out = (attn v) @ w_proj + b

Sharding: data-parallel over windows, 512 windows per core on 8 cores.

Dataflow (per superchunk of 4 windows = 256 tokens; matmul operands bf16):
  x downcast on GPSIMD; xT via PE bf16 transpose; qT/kT weight-stationary (512 cols)
  v[m, c-perm] head-parity packed (512 cols, no duplication)
  attnT[m, n] per (window, head) (1024 cols)
  P = exp(attnT) (ACT) * exp(bias) (DVE bf16)
  av[n, hd] with P^T stationary (512 cols) + softmax sums as 1-col matmuls (16)
  av_sb = av * recip(s) broadcast (DVE, fused evacuation)
  avT via 8 PE [64,64] transposes (512 cols); y = avT @ wp_perm (256 cols)
  b_proj added on HOST after gather.

The per-superchunk stages are emitted SOFTWARE-PIPELINED with a deep skew;
at iteration i the emitted stage instances are
  ATT(i-2) DC(i+3) T(i+2) XTE(i+1) QKV(i) QKE/VE(i-1) EXP(i-3) BIAS(i-4)
  AVS(i-6) NORM(i-7) AVTE(i-9) AVT(i-8) PROJ(i-10) YE(i-11) RECIP(i-6)
so every cross-engine dependency (except the deliberate tail RECIP) is at
least one iteration old and no engine head-of-line blocks. Engine split:
PE matmuls ~1.50us/iter, ACT {qk-evac, exp, y-evac} ~1.63us, DVE {xt/v/avT
evacs, recip, norm} ~1.43us, Pool (GPSIMD, no PSUM access!) {x downcast,
bias mul} ~1.56us, DMA ~0.75us. x is group-prefetched ~1.5 groups ahead.

PSUM-bank drain rule respected: no two in-flight matmuls with different
tile_position row-groups and the same column-group share a PSUM bank.
"""

import sys

sys.path.insert(0, "/opt/trn_rl_repo")

import numpy as np
import ml_dtypes

WS = 8
H = 4
DIM = 128
N = WS * WS  # 64 tokens per window
HD = DIM // H  # 32
B = 4096
NCORES = 8
BC = B // NCORES  # 512 windows per core
ROWS = BC * N  # 32768 rows per core

SC_W = 4  # windows per superchunk
SC_ROWS = SC_W * N  # 256
N_SC = BC // SC_W  # 128 superchunks
GROUP = 4  # superchunks per input DMA group
N_G = N_SC // GROUP  # 32 groups

bf16 = ml_dtypes.bfloat16

# channel permutation used for wv cols / wp rows / avT partitions:
# p = 64*hp + 32*hh + d  <->  c = 32*(2*hh + hp) + d   (head h = 2*hh + hp)
_PERM = np.array(
    [32 * (2 * ((p % 64) // 32) + p // 64) + p % 32 for p in range(DIM)], dtype=np.int64
)


def _rel_pos_index(ws: int) -> np.ndarray:
    coords = np.stack(np.meshgrid(np.arange(ws), np.arange(ws), indexing="ij"))
    flat = coords.reshape(2, -1)
    rel = flat[:, :, None] - flat[:, None, :]
    rel = rel.transpose(1, 2, 0).astype(np.int64)
    rel[..., 0] += ws - 1
    rel[..., 1] += ws - 1
    rel[..., 0] *= 2 * ws - 1
    return rel.sum(-1)


_BUILT = {}


def _build_program(n_groups=N_G, compile=True, stage=9, bufs=None):
    """Build + compile the single-core Bass program (same program runs SPMD
    on all cores). stage < 9 truncates the pipeline for bisection: that
    stage's intermediate is written to y instead.

    See module docstring for the stage skew; measured 263775 ns/core in
    TimelineSim (baseline kernel: 485740 ns)."""
    bufs = dict(
        dict(sb1=7, sb2=5, qk=2, xp=4, yp=6, att_first=1, hoist=1),
        **(bufs or {}),
    )
    key = (n_groups, compile, stage, tuple(sorted(bufs.items())))
    if key in _BUILT:
        return _BUILT[key]
    n_sc = n_groups * GROUP

    from contextlib import ExitStack

    import concourse.tile as tile
    from concourse import bacc, mybir
    from concourse.masks import make_identity

    f32 = mybir.dt.float32
    bf = mybir.dt.bfloat16
    EXP = mybir.ActivationFunctionType.Exp

    nc = bacc.Bacc("TRN2", target_bir_lowering=False, debug=False, enable_asserts=False)

    x_d = nc.dram_tensor("x", [n_sc * SC_ROWS, DIM], f32, kind="ExternalInput").ap()
    wq_d = nc.dram_tensor("wq_bf", [DIM, DIM], bf, kind="ExternalInput").ap()
    wk_d = nc.dram_tensor("wk_bf", [DIM, DIM], bf, kind="ExternalInput").ap()
    wv_d = nc.dram_tensor("wvp_bf", [DIM, DIM], bf, kind="ExternalInput").ap()
    wp_d = nc.dram_tensor("wpp_bf", [DIM, DIM], bf, kind="ExternalInput").ap()
    # eb[64*hp + m, 256*hb + 64*w + n] = exp(rel_bias[2*hb+hp][n, m]), window-tiled
    eb_d = nc.dram_tensor("expbias_bf", [DIM, 2 * H * N], bf, kind="ExternalInput").ap()
    y_d = nc.dram_tensor("y", [n_sc * SC_ROWS, DIM], f32, kind="ExternalOutput").ap()

    with tile.TileContext(nc) as tc, ExitStack() as ctx:
        consts = ctx.enter_context(tc.tile_pool(name="consts", bufs=1))
        xp = ctx.enter_context(tc.tile_pool(name="xp", bufs=bufs["xp"]))
        sb1 = ctx.enter_context(tc.tile_pool(name="sb1", bufs=bufs["sb1"]))
        sb2 = ctx.enter_context(tc.tile_pool(name="sb2", bufs=bufs["sb2"]))
        yp = ctx.enter_context(tc.tile_pool(name="yp", bufs=bufs["yp"]))

        # PSUM (8 banks of 2KB/partition):
        #   at  [128,1024] f32, manual parity halves        2 banks
        #   qk  [128,512]  f32 x2 bufs                      2 banks
        #   v   [128,256]  f32 x2                           1 bank
        #   av  [128,256]  f32 x2                           1 bank
        #   y   [128,256]  f32 x2                           1 bank
        #   sm: xt_ps bf16 x2 + avT_ps bf16 x1 + s f32 x2   1 bank
        ps_at = ctx.enter_context(tc.tile_pool(name="ps_at", bufs=1, space="PSUM"))
        ps_qk = ctx.enter_context(tc.tile_pool(name="ps_qk", bufs=bufs["qk"], space="PSUM"))
        ps_v = ctx.enter_context(tc.tile_pool(name="ps_v", bufs=1, space="PSUM"))
        ps_av = ctx.enter_context(tc.tile_pool(name="ps_av", bufs=1, space="PSUM"))
        ps_y = ctx.enter_context(tc.tile_pool(name="ps_y", bufs=1, space="PSUM"))
        ps_sm = ctx.enter_context(tc.tile_pool(name="ps_sm", bufs=1, space="PSUM"))

        # constants
        wq = consts.tile([DIM, DIM], bf)
        wk = consts.tile([DIM, DIM], bf)
        wvp = consts.tile([DIM, DIM], bf)
        wpp = consts.tile([DIM, DIM], bf)
        eb = consts.tile([DIM, 2 * H * N], bf)
        nc.sync.dma_start(wq[:], wq_d)
        nc.sync.dma_start(wk[:], wk_d)
        nc.sync.dma_start(wvp[:], wv_d)
        nc.sync.dma_start(wpp[:], wp_d)
        nc.sync.dma_start(eb[:], eb_d)

        identf = consts.tile([DIM, DIM], f32)
        make_identity(nc, identf[:])
        ident = consts.tile([DIM, DIM], bf)
        nc.gpsimd.tensor_copy(ident[:], identf[:])
        ones1 = consts.tile([DIM, 1], bf)
        nc.vector.memset(ones1[:], 1.0)

        live = {}
        cur_ps = {}
        cur_ysb = [None]

        def x_load(g):
            x_g = xp.tile([128, 2 * GROUP, DIM], f32, tag="x_g")
            r0 = g * GROUP * SC_ROWS
            nc.sync.dma_start(
                out=x_g[:],
                in_=x_d[r0 : r0 + GROUP * SC_ROWS, :].rearrange(
                    "(t p) c -> p t c", p=128
                ),
            )
            return x_g

        def st_DC(i):
            """Pool: downcast x -> bf16."""
            sc = live[i]
            so = i % GROUP
            x_bf = sb1.tile([128, 2, DIM], bf, tag="x_bf")
            nc.gpsimd.tensor_copy(x_bf[:], sc["x_g"][:, 2 * so : 2 * so + 2, :])
            sc["x_bf"] = x_bf

        def st_T(i):
            """PE: bf16 transpose -> xt_ps."""
            sc = live[i]
            xt_ps = cur_ps["sm"][:, 128 * (i % 2) : 128 * (i % 2) + 128].bitcast(bf)
            for t in range(2):
                nc.tensor.matmul(
                    xt_ps[:, t * 128 : (t + 1) * 128],
                    sc["x_bf"][:, t, :],
                    ident[:],
                    is_transpose=True,
                    start=True,
                    stop=True,
                )
            sc["xt_ps"] = xt_ps

        def st_XTE(i):
            """DVE: xt evac (bf16 2x)."""
            sc = live[i]
            xt = sb1.tile([128, SC_ROWS], bf, tag="xt")
            nc.vector.tensor_copy(xt[:], sc["xt_ps"][:])
            sc["xt"] = xt

        def st_QKV(i):
            sc = live[i]
            xt = sc["xt"]
            qk_ps = ps_qk.tile([128, 2 * SC_ROWS], f32, tag="qk_ps")
            nc.tensor.matmul(qk_ps[:, 0:SC_ROWS], wq[:], xt[:], start=True, stop=True)
            nc.tensor.matmul(
                qk_ps[:, SC_ROWS : 2 * SC_ROWS], wk[:], xt[:], start=True, stop=True
            )
            v_ps = ps_v.tile([128, 2 * SC_ROWS], f32, tag="v_ps", name="v_ps")[
                :, 256 * (i % 2) : 256 * (i % 2) + SC_ROWS
            ]
            for w in range(SC_W):
                for hp in range(2):
                    nc.tensor.matmul(
                        v_ps[64 * hp : 64 * hp + 64, 64 * w : 64 * (w + 1)],
                        xt[:, 64 * w : 64 * (w + 1)],
                        wvp[:, 64 * hp : 64 * hp + 64],
                        tile_position=(0, 64 * hp),
                        start=True,
                        stop=True,
                    )
            sc["qk_ps"] = qk_ps
            sc["v_ps"] = v_ps

        def st_QKE(i):
            sc = live[i]
            qk = sb1.tile([128, 2 * SC_ROWS], bf, tag="qk")
            nc.scalar.copy(qk[:], sc["qk_ps"][:])
            sc["qk"] = qk

        def st_VE(i):
            sc = live[i]
            vd = sb1.tile([128, SC_ROWS], bf, tag="vd")
            nc.vector.tensor_copy(vd[:], sc["v_ps"][:])
            sc["vd"] = vd

        def st_ATT(i):
            """PE attnT (at_ps halves alternate by sc parity); ACT exp; Pool bias."""
            sc = live[i]
            qk = sc["qk"]
            at_ps = ps_at.tile([128, 1024], f32, tag="at_ps")
            off = 256 * (i % 2)
            for w in range(SC_W):
                for h in range(H):
                    hp, hb = h % 2, h // 2
                    nc.tensor.matmul(
                        at_ps[
                            64 * hp : 64 * hp + 64,
                            512 * hb + off + 64 * w : 512 * hb + off + 64 * (w + 1),
                        ],
                        qk[32 * h : 32 * h + 32, SC_ROWS + 64 * w : SC_ROWS + 64 * (w + 1)],
                        qk[32 * h : 32 * h + 32, 64 * w : 64 * (w + 1)],
                        tile_position=(32 * h, 64 * hp),
                        start=True,
                        stop=True,
                    )
            sc["at_ps"] = at_ps
            sc["at_off"] = off

        def st_EXP(i):
            sc = live[i]
            at_ps, off = sc["at_ps"], sc["at_off"]
            pt = sb2.tile([128, 2 * SC_ROWS], bf, tag="pt")
            nc.scalar.activation(
                pt[:].rearrange("p (b c) -> p b c", b=2),
                at_ps[:].rearrange("p (b c) -> p b c", b=2)[:, :, off : off + SC_ROWS],
                EXP,
            )
            sc["pt"] = pt

        def st_BIAS(i):
            sc = live[i]
            pb = sb2.tile([128, 2 * SC_ROWS], bf, tag="pb")
            if bufs.get("bias_split"):
                nc.vector.tensor_mul(
                    pb[:, 0:SC_ROWS], sc["pt"][:, 0:SC_ROWS], eb[:, 0:SC_ROWS]
                )
                nc.gpsimd.tensor_mul(
                    pb[:, SC_ROWS : 2 * SC_ROWS],
                    sc["pt"][:, SC_ROWS : 2 * SC_ROWS],
                    eb[:, SC_ROWS : 2 * SC_ROWS],
                )
            else:
                eng = nc.vector if bufs.get("bias_dve") else nc.gpsimd
                eng.tensor_mul(pb[:], sc["pt"][:], eb[:, 0 : 2 * SC_ROWS])
            sc["pb"] = pb

        def st_AVS(i):
            """PE: av[n, hd] + sums (P^T stationary)."""
            sc = live[i]
            pb, vd = sc["pb"], sc["vd"]
            av_ps = ps_av.tile([128, 2 * SC_ROWS], f32, tag="av_ps", name="av_ps")[
                :, 256 * (i % 2) : 256 * (i % 2) + SC_ROWS
            ]
            s_ps = cur_ps["sm"][:, 384 + 8 * (i % 2) : 384 + 8 * (i % 2) + 8]
            # sums first: RECIP's dependency completes before the slower av
            # matmuls, pulling the recip->norm->next-AVS chain earlier.
            for w in range(SC_W):
                for h in range(H):
                    hp, hh = h % 2, h // 2
                    nc.tensor.matmul(
                        s_ps[64 * hp : 64 * hp + 64, 2 * w + hh : 2 * w + hh + 1],
                        pb[
                            64 * hp : 64 * hp + 64,
                            256 * hh + 64 * w : 256 * hh + 64 * (w + 1),
                        ],
                        ones1[64 * hp : 64 * hp + 64, :],
                        tile_position=(64 * hp, 64 * hp),
                        start=True,
                        stop=True,
                    )
            for w in range(SC_W):
                for h in range(H):
                    hp, hh = h % 2, h // 2
                    nc.tensor.matmul(
                        av_ps[
                            64 * hp : 64 * hp + 64,
                            64 * w + 32 * hh : 64 * w + 32 * hh + 32,
                        ],
                        pb[
                            64 * hp : 64 * hp + 64,
                            256 * hh + 64 * w : 256 * hh + 64 * (w + 1),
                        ],
                        vd[
                            64 * hp : 64 * hp + 64,
                            64 * w + 32 * hh : 64 * w + 32 * hh + 32,
                        ],
                        tile_position=(64 * hp, 64 * hp),
                        start=True,
                        stop=True,
                    )
            sc["av_ps"] = av_ps
            sc["s_ps"] = s_ps

        def st_RECIP(i):
            sc = live[i]
            rf = sb2.tile([128, 8], f32, tag="rf")
            nc.vector.reciprocal_approx_fast(out=rf[:], in_=sc["s_ps"][:])
            sc["rf"] = rf

        def st_NORM(i):
            sc = live[i]
            av_sb = sb2.tile([128, SC_ROWS], bf, tag="av_sb")
            rf_b = sc["rf"][:].unsqueeze(-1).broadcast_to((128, 8, 32))
            nc.vector.tensor_mul(
                av_sb[:].rearrange("p (k d) -> p k d", k=8),
                sc["av_ps"][:].rearrange("p (k d) -> p k d", k=8),
                rf_b,
            )
            sc["av_sb"] = av_sb

        def st_AVT(i):
            sc = live[i]
            av_sb = sc["av_sb"]
            avT_ps = cur_ps["sm"][:, 256:384].bitcast(bf)
            for w in range(SC_W):
                for hp in range(2):
                    nc.tensor.matmul(
                        avT_ps[64 * hp : 64 * hp + 64, 64 * w : 64 * (w + 1)],
                        av_sb[64 * hp : 64 * hp + 64, 64 * w : 64 * (w + 1)],
                        ident[64 * hp : 64 * hp + 64, 64 * hp : 64 * hp + 64],
                        is_transpose=True,
                        tile_position=(64 * hp, 64 * hp),
                        start=True,
                        stop=True,
                    )
            sc["avT_ps"] = avT_ps

        def st_AVTE(i):
            sc = live[i]
            avt = sb1.tile([128, SC_ROWS], bf, tag="avt")
            nc.vector.tensor_copy(avt[:], sc["avT_ps"][:])
            sc["avt"] = avt

        def st_PROJ(i):
            sc = live[i]
            avt = sc["avt"]
            y_tile = ps_y.tile([128, 4 * DIM], f32, tag="y_ps", name="y_ps")
            y_ps = y_tile[:, 256 * (i % 2) : 256 * (i % 2) + 2 * DIM]
            sc["y_tile"] = y_tile
            for j in range(2):
                nc.tensor.matmul(
                    y_ps[:, 128 * j : 128 * (j + 1)],
                    avt[:, 128 * j : 128 * (j + 1)],
                    wpp[:],
                    start=True,
                    stop=True,
                )
            sc["y_ps"] = y_ps

        def st_YE(i):
            sc = live[i]
            half = i % 2
            if half == 0:
                cur_ysb[0] = yp.tile([128, 4, DIM], f32, tag="y_sb", name="y_sb")
            y_sb = cur_ysb[0]
            nc.scalar.copy(
                y_sb[:, 2 * half : 2 * half + 2, :].rearrange("p a b -> p (a b)"),
                sc["y_ps"][:],
            )
            if half == 1:
                r0 = (i - 1) * SC_ROWS
                nc.sync.dma_start(
                    out=y_d[r0 : r0 + 2 * SC_ROWS, :].rearrange(
                        "(t p) c -> p t c", p=128
                    ),
                    in_=y_sb[:],
                )

        def dump(i, src_ap):
            """Bisect helper (SBUF src only): route [128, 256] to y rows of sc i."""
            half = i % 2
            if half == 0:
                cur_ysb[0] = yp.tile([128, 4, DIM], f32, tag="y_sb", name="y_sb")
            y_sb = cur_ysb[0]
            nc.gpsimd.tensor_copy(
                y_sb[:, 2 * half : 2 * half + 2, :].rearrange("p a b -> p (a b)"),
                src_ap,
            )
            if half == 1:
                r0 = (i - 1) * SC_ROWS
                nc.sync.dma_start(
                    out=y_d[r0 : r0 + 2 * SC_ROWS, :].rearrange(
                        "(t p) c -> p t c", p=128
                    ),
                    in_=y_sb[:],
                )

        def run_iter(it, last):
            cur_ps["sm"] = ps_sm.tile([128, 512], f32, tag="sm", name="sm")
            td = 1 if bufs.get("tail_deep") else 0
            bo = 5 if bufs.get("bias_deep") else 4

            def S(cond, fn, *a):
                if cond:
                    fn(*a)

            if bufs.get("hoist2"):
                # consumers-first: every op whose deps are >=1 iter old is
                # emitted before this iteration's producers.
                S(0 <= it - 9 - td <= last and stage >= 5, st_AVTE, it - 9 - td)
                S(0 <= it - 7 <= last and stage >= 4, st_NORM, it - 7)
                S(0 <= it + 1 <= last, st_XTE, it + 1)
                S(0 <= it - 1 <= last and stage >= 2, st_QKE, it - 1)
                S(0 <= it - 3 <= last and stage >= 3, st_EXP, it - 3)
                S(0 <= it - 11 - td <= last and stage >= 5, st_YE, it - 11 - td)
                S(0 <= it - bo <= last and stage >= 3, st_BIAS, it - bo)
                if it + 7 <= last + 4 and (it + 7) % GROUP == 0:
                    g = (it + 7) // GROUP
                    if 0 < g < n_groups:
                        live_g = x_load(g)
                        for k in range(GROUP):
                            live.setdefault(g * GROUP + k, {})["x_g"] = live_g
                S(0 <= it + 3 <= last, st_DC, it + 3)
                S(0 <= it - 2 <= last and stage >= 3, st_ATT, it - 2)
                S(0 <= it + 2 <= last, st_T, it + 2)
                S(0 <= it <= last and stage >= 2, st_QKV, it)
                S(0 <= it - 6 <= last and stage >= 4, st_AVS, it - 6)
                S(0 <= it - 8 - td <= last and stage >= 5, st_AVT, it - 8 - td)
                S(0 <= it - 10 - td <= last and stage >= 5, st_PROJ, it - 10 - td)
                S(0 <= it - 1 <= last and stage >= 2, st_VE, it - 1)
                S(0 <= it - 6 <= last and stage >= 4, st_RECIP, it - 6)
                if stage < 2 and 0 <= it + 1 <= last:
                    dump(it + 1, live[it + 1]["xt"][:])
                if stage == 2 and 0 <= it - 1 <= last:
                    dump(it - 1, live[it - 1]["qk"][:, 0:SC_ROWS])
                if stage == 3 and 0 <= it - bo <= last:
                    dump(it - bo, live[it - bo]["pb"][:, 0:SC_ROWS])
                if stage == 4 and 0 <= it - 7 <= last:
                    dump(it - 7, live[it - 7]["av_sb"][:])
                if 0 <= it - 12 - td <= last and stage >= 5:
                    live.pop(it - 12 - td)
                return

            if bufs.get("hoist"):
                td0 = 1 if bufs.get("tail_deep") else 0
                if 0 <= it - 9 - td0 <= last and stage >= 5:
                    st_AVTE(it - 9 - td0)
                if 0 <= it - 7 <= last and stage >= 4:
                    st_NORM(it - 7)
            if bufs.get("att_first") and 0 <= it - 2 <= last and stage >= 3:
                st_ATT(it - 2)
            if it + 7 <= last + 4 and (it + 7) % GROUP == 0:
                g = (it + 7) // GROUP
                if 0 < g < n_groups:
                    live_g = x_load(g)
                    for k in range(GROUP):
                        live.setdefault(g * GROUP + k, {})["x_g"] = live_g
            if 0 <= it + 3 <= last:
                st_DC(it + 3)
            if 0 <= it + 2 <= last:
                st_T(it + 2)
            if 0 <= it + 1 <= last:
                st_XTE(it + 1)
                if stage < 2:
                    dump(it + 1, live[it + 1]["xt"][:])
            if 0 <= it <= last and stage >= 2:
                st_QKV(it)
            if 0 <= it - 1 <= last and stage >= 2:
                st_QKE(it - 1)
                st_VE(it - 1)
                if stage < 3:
                    dump(it - 1, live[it - 1]["qk"][:, 0:SC_ROWS])
            if not bufs.get("att_first") and 0 <= it - 2 <= last and stage >= 3:
                st_ATT(it - 2)
            if 0 <= it - 3 <= last and stage >= 3:
                st_EXP(it - 3)
            if 0 <= it - bo <= last and stage >= 3:
                st_BIAS(it - bo)
                if stage < 4:
                    dump(it - bo, live[it - bo]["pb"][:, 0:SC_ROWS])
            if 0 <= it - 6 <= last and stage >= 4:
                st_AVS(it - 6)
            if 0 <= it - 7 <= last and stage >= 4:
                if not bufs.get("hoist"):
                    st_NORM(it - 7)
                if stage < 5:
                    dump(it - 7, live[it - 7]["av_sb"][:])
            if not bufs.get("hoist") and 0 <= it - 9 - td <= last and stage >= 5:
                st_AVTE(it - 9 - td)
            if 0 <= it - 8 - td <= last and stage >= 5:
                st_AVT(it - 8 - td)
            if 0 <= it - 10 - td <= last and stage >= 5:
                st_PROJ(it - 10 - td)
            if 0 <= it - 11 - td <= last and stage >= 5:
                st_YE(it - 11 - td)
                live.pop(it - 11 - td)
            if not bufs.get("recip7") and 0 <= it - 6 <= last and stage >= 4:
                st_RECIP(it - 6)

        g0 = x_load(0)
        for k in range(GROUP):
            live.setdefault(k, {})["x_g"] = g0
        for it in range(-3, n_sc + 12 + (1 if bufs.get("tail_deep") else 0)):
            run_iter(it, n_sc - 1)

    if compile:
        nc.compile()
    _BUILT[key] = nc
    return nc


def _host_prep(w_qkv, w_proj, bias_table):
    """Precompute replicated small tensors (channel-permuted for the kernel)."""
    scale = HD**-0.5
    wq = (w_qkv[:, :DIM] * scale).astype(bf16)
    wk = w_qkv[:, DIM : 2 * DIM].astype(bf16)
    wv = w_qkv[:, 2 * DIM :]
    wvp = np.ascontiguousarray(wv[:, _PERM]).astype(bf16)
    wpp = np.ascontiguousarray(w_proj[_PERM, :]).astype(bf16)

    rel = _rel_pos_index(WS)  # [N, N]
    rel_bias = bias_table[rel.reshape(-1)].reshape(N, N, H).transpose(2, 0, 1)  # [h,n,m]
    ebv = np.exp(rel_bias).astype(np.float32)  # [h, n, m]
    # eb[64*hp + m, 256*hb + 64*w + n] = ebv[2*hb + hp][n, m]
    eb = np.zeros((DIM, 512), np.float32)
    for hb in range(2):
        for hp in range(2):
            h = 2 * hb + hp
            blk = ebv[h].T  # [m, n]
            for w in range(SC_W):
                eb[
                    64 * hp : 64 * hp + 64, 256 * hb + 64 * w : 256 * hb + 64 * (w + 1)
                ] = blk
    eb = eb.astype(bf16)
    return wq, wk, wvp, wpp, eb


def run(x, w_qkv, w_proj, b_proj, bias_table, trace=False, **trace_kwargs):
    """Run on 8 NeuronCores. Returns (y, BassKernelResults)."""
    from concourse import bass_utils

    x = np.asarray(x, dtype=np.float32)
    w_qkv = np.asarray(w_qkv, dtype=np.float32)
    w_proj = np.asarray(w_proj, dtype=np.float32)
    b_proj = np.asarray(b_proj, dtype=np.float32)
    bias_table = np.asarray(bias_table, dtype=np.float32)

    wq, wk, wvp, wpp, eb = _host_prep(w_qkv, w_proj, bias_table)
    nc = _build_program()

    xs = x.reshape(B * N, DIM)
    in_maps = []
    for c in range(NCORES):
        in_maps.append(
            {
                "x": np.ascontiguousarray(xs[c * ROWS : (c + 1) * ROWS]),
                "wq_bf": wq,
                "wk_bf": wk,
                "wvp_bf": wvp,
                "wpp_bf": wpp,
                "expbias_bf": eb,
            }
        )

    res = bass_utils.run_bass_kernel_spmd(
        nc, in_maps, core_ids=list(range(NCORES)), trace=trace, **trace_kwargs
    )
    y = np.concatenate([res.results[c]["y"] for c in range(NCORES)], axis=0)
    y = y + b_proj[None, :]
    return y.reshape(B, N, DIM), res


def kernel(x, w_qkv, w_proj, b_proj, bias_table):
    y, _ = run(x, w_qkv, w_proj, b_proj, bias_table)
    return y


if __name__ == "__main__":
    sys.path.insert(0, "/root/problem")
    import reference

    inputs = {k: np.asarray(v) for k, v in reference.setup_inputs().items()}
    out = kernel(**inputs)
    exp = np.asarray(reference.reference(**inputs))
    err = np.abs(out - exp)
    print("abs max err:", err.max(), "scale-rel:", err.max() / np.abs(exp).max())
